# revision 6
# baseline (speedup 1.0000x reference)
"""GQA attention (RoPE + causal softmax + o_proj) on 8 Trainium2 NeuronCores.

Sharding: core = b*4 + g where b = batch (2), g = head-group (4).
Each core handles 8 query heads (global 8g..8g+7) and their 2 KV heads
(2g, 2g+1) for one batch element, producing a partial o_proj output
(contraction over its 512 of the 2048 hd dims). The host sums the 4
partials per batch element (o_part is bf16; host upcasts).

Per-core layout/schedule (all matmul operands bf16, fp32 PSUM accum):
  - Inputs are host-packed into 4 flat [128, N] tensors so the whole
    input load is 5 DMA instructions, issued from sync/scalar/vector
    queues in parallel (DMA issue costs ~0.7us of the issuing engine's
    queue REGARDLESS of size, so many small input DMAs serialize the
    sync queue and block the rope-swap DMAs behind them).
  - q^T/k^T built per 128-row chunk pairing heads (i, i+4); scores are
    computed transposed (S^T[k,q]) as two row-tiled K=64 matmuls that
    run concurrently in the PE array.
  - AV stationary vnat[kb] = [v0(0:64) | 1 | 0*63 | v1(128:192)]:
      av0 = vnat[:,0:65].T  @ pt0 -> v0 at partitions 0:64, den0 at 64
      av1 = vnat[:,64:192].T @ pt1 -> den1 at partition 0, v1 at 64:128
    so attnT rows 64:128 are written lane-aligned (no SBUF->SBUF DMA)
    and den1 feeds reciprocal/broadcast without a partition move.
  - At pg end avs are evicted to SBUF immediately (frees the 2 "av"
    PSUM banks for the next pg), then den->recip->broadcast->mul chains
    run off SBUF (muls on GPSIMD; recip/evict on DVE).
  - Schedule: proj(0) dense, then attention chunk c interleaves
    proj(c+1) + o_proj(c-1) units as PE filler (own "fil" PSUM slots)
    so ScalarE exp (the per-kb rate limiter) starts early and PE never
    drains. PSUM: st 2x2 + av 2 + fil 2 = 8 banks.
  - Engine balance: exp exclusively on ACT; PSUM reads (qraw/t1/
    evictions) on DVE; rope t2+add, diag-mask half, norm muls on GPSIMD.
"""

import numpy as np
import ml_dtypes
from contextlib import ExitStack

import concourse.mybir as mybir
from concourse import bacc
from concourse.tile import TileContext
from concourse.bass_utils import run_bass_kernel_spmd

BF16 = mybir.dt.bfloat16
F32 = mybir.dt.float32
NP_BF16 = ml_dtypes.bfloat16

HID = 2048
D = 64
H = 32           # global query heads
KV = 8           # global kv heads
B = 2
P = 128
SC = 512         # q-chunk width (also matmul free dim / PSUM bank)

_CACHE = {}


def build_nc(S):
    assert S % SC == 0
    NHID = HID // P       # hid chunks (16)
    NSB = S // P          # 128-row s-blocks
    NSC = S // SC         # 512-col s-chunks
    QCH = 4               # q chunk-pairs
    EXP = mybir.ActivationFunctionType.Exp

    nc = bacc.Bacc("TRN2", target_bir_lowering=False, debug=False)
    # host-packed flat inputs (see prep_core_inputs for layouts)
    xTp = nc.dram_tensor("xTp", [P, NHID * S], BF16, kind="ExternalInput")
    wqkvp = nc.dram_tensor("wqkvp", [P, NHID * 768], BF16, kind="ExternalInput")
    csm = nc.dram_tensor("csm", [P, 2 * S + P], BF16, kind="ExternalInput")
    wop = nc.dram_tensor("wop", [P, 4 * HID], BF16, kind="ExternalInput")
    o_part = nc.dram_tensor("o_part", [S, HID], BF16, kind="ExternalOutput")

    with TileContext(nc) as tc, ExitStack() as ctx:
        res = ctx.enter_context(tc.tile_pool(name="res", bufs=1))
        rope = ctx.enter_context(tc.tile_pool(name="rope", bufs=2))
        ptp = ctx.enter_context(tc.tile_pool(name="ptp", bufs=6))
        nrm = ctx.enter_context(tc.tile_pool(name="nrm", bufs=1))
        obp = ctx.enter_context(tc.tile_pool(name="obp", bufs=2))
        psum = ctx.enter_context(tc.tile_pool(name="psum", bufs=1, space="PSUM"))

        # ---- input DMA: 5 big transfers on 3 independent queues ----
        xtall = res.tile([P, NHID * S], BF16, tag="xtall", name="xtall")
        nc.sync.dma_start(out=xtall[:, 0:NHID * SC], in_=xTp[:, 0:NHID * SC])
        wqall = res.tile([P, NHID * 768], BF16, tag="wqall", name="wqall")
        nc.scalar.dma_start(out=wqall, in_=wqkvp[:, :])
        csm_t = res.tile([P, 2 * S + P], BF16, tag="csm", name="csm")
        nc.scalar.dma_start(out=csm_t, in_=csm[:, :])
        nc.scalar.dma_start(
            out=xtall[:, NHID * SC:], in_=xTp[:, NHID * SC:]
        )
        woall = res.tile([P, 4 * HID], BF16, tag="woall", name="woall")
        nc.scalar.dma_start(out=woall, in_=wop[:, :])

        # packed-layout views
        def xtv(h, s):      # x^T [hid chunk h, s-chunk s] -> [128, 512]
            base = (s * NHID + h) * SC
            return xtall[:, base:base + SC]

        def xtvb(h, sb):    # x^T [hid chunk h, s-block sb] -> [128, 128]
            base = ((sb // 4) * NHID + h) * SC + (sb % 4) * P
            return xtall[:, base:base + P]

        def wqv(h, m):      # wqkv [hid chunk h, col chunk m] (m=4 -> k)
            return wqall[:, h * 768 + m * P: h * 768 + (m + 1) * P]

        def wvv(h):         # wv [hid chunk h] -> [128, 128]
            return wqall[:, h * 768 + 640: h * 768 + 768]

        cos_sb = csm_t[:, 0:S]
        sin_sb = csm_t[:, S:2 * S]
        mask_sb = csm_t[:, 2 * S:2 * S + P]

        def wov(i, n):      # wo [hd chunk i, hid cols n*SC..] -> [128, 512]
            return woall[:, i * HID + n * SC: i * HID + (n + 1) * SC]

        # chunks 0-3: q head pairs (i, i+4); chunk 4: k (kv0 rows 0-63, kv1 64-127)
        qkrot = []
        for m in range(5):
            t = res.tile([P, S], BF16, tag=f"qkrot{m}", name=f"qkrot{m}")
            qkrot.append(t)
        # v tiles [128, 192]: [v0(0:64) | 1 | 0*63 | v1(128:192)]
        vnat = [res.tile([P, 192], BF16, tag=f"vnat{sb}", name=f"vnat{sb}")
                for sb in range(NSB)]
        attnT = []
        for i in range(QCH):
            t = res.tile([P, S], BF16, tag=f"attnT{i}", name=f"attnT{i}")
            attnT.append(t)

        def gen_o_chunk(c):
            for qb in range(4 * c, 4 * c + 4):
                ob = obp.tile([P, HID], BF16, tag="ob", name="ob")
                for n in range(HID // SC):
                    po = psum.tile([P, SC], F32, tag="fil", bufs=2, name="po")
                    for i in range(QCH):
                        nc.tensor.matmul(
                            po,
                            lhsT=attnT[i][:, qb * P:(qb + 1) * P],
                            rhs=wov(i, n),
                            start=(i == 0),
                            stop=(i == QCH - 1),
                        )
                    nc.vector.tensor_copy(ob[:, n * SC:(n + 1) * SC], po)
                    yield
                nc.sync.dma_start(out=o_part[qb * P:(qb + 1) * P, :], in_=ob)

        def gen_proj_schunk(s):
            """Emit s-chunk s projections + RoPE + v as units (yields).

            Matmul groups stay consecutive; eviction units only follow
            completed groups. Rope eviction split: qraw/t1 on DVE (the
            PSUM readers), t2 + final add on GPSIMD so the DVE FIFO
            never head-of-line-blocks on the qswp DMA chain."""
            sl = slice(s * SC, (s + 1) * SC)
            for m in (4, 0, 1, 2, 3):
                ps = psum.tile([P, SC], F32, tag="fil", bufs=2, name="ps_proj")
                for h0 in (0, 8):
                    for h in range(h0, h0 + 8):
                        nc.tensor.matmul(
                            ps,
                            lhsT=wqv(h, m),
                            rhs=xtv(h, s),
                            start=(h == 0),
                            stop=(h == NHID - 1),
                        )
                    yield
                # rotate_half operand: engines are lane-locked, so the
                # +-32-partition swap must go through DMA (SBUF->SBUF);
                # one DMA per direction with a 3-level partition pattern
                qraw = rope.tile([P, SC], BF16, tag="qraw", bufs=2, name="qraw")
                nc.vector.tensor_copy(qraw, ps)
                qswp = rope.tile([P, SC], BF16, tag="qswp", bufs=2, name="qswp")
                for dst, src in ((0, 32), (32, 0), (64, 96), (96, 64)):
                    nc.sync.dma_start(
                        out=qswp[dst:dst + 32, :], in_=qraw[src:src + 32, :]
                    )
                t1 = rope.tile([P, SC], BF16, tag="t1", bufs=2, name="t1")
                nc.vector.tensor_mul(t1, ps, cos_sb[:, sl])
                t2 = rope.tile([P, SC], BF16, tag="t2", bufs=2, name="t2")
                nc.gpsimd.tensor_mul(t2, qswp, sin_sb[:, sl])
                nc.gpsimd.tensor_add(qkrot[m][:, sl], t1, t2)
                yield
            for sb in range(4 * s, 4 * s + 4):
                t = vnat[sb]
                nc.gpsimd.memset(t[:, 64:65], 1.0)
                nc.gpsimd.memset(t[:, 65:128], 0.0)
                pv = psum.tile([P, 128], F32, tag="fil", bufs=2, name="ps_v")
                for h in range(NHID):
                    nc.tensor.matmul(
                        pv,
                        lhsT=xtvb(h, sb),
                        rhs=wvv(h),
                        start=(h == 0),
                        stop=(h == NHID - 1),
                    )
                yield
                nc.vector.tensor_copy(t[:, 0:64], pv[:, 0:64])
                nc.vector.tensor_copy(t[:, 128:192], pv[:, 64:128])
                yield

        def interleave(*gens):
            gens = [g for g in gens if g is not None]
            i = 0
            while gens:
                g = gens[i % len(gens)]
                try:
                    next(g)
                except StopIteration:
                    gens.remove(g)
                    continue
                yield
                i += 1

        # ---- proj chunk 0 dense (nothing to overlap with yet) ----
        with nc.named_scope("projA0"):
            for _ in gen_proj_schunk(0):
                pass

        # ---- attention chunks; proj(c+1) and o(c-1) drip into the kb
        # loop as PE fill work while ScalarE streams exps ----
        for c in range(NSC):
          with nc.named_scope(f"attn_c{c}"):
            q0 = c * SC
            nkb = 4 * c + 4
            filler = interleave(
                gen_proj_schunk(c + 1) if c + 1 < NSC else None,
                gen_o_chunk(c - 1) if c >= 1 else None,
            )
            n_units = (23 if c + 1 < NSC else 0) + (16 if c >= 1 else 0)
            total_iters = 4 * nkb
            it = 0
            spent = 0
            for pg in (0, 1, 2, 3):
                av0 = psum.tile([P, SC], F32, tag="av", bufs=2, name="av0")
                av1 = psum.tile([P, SC], F32, tag="av", bufs=2, name="av1")
                hp = pg

                def emit_av(kb, pt, vs):
                    nc.tensor.matmul(
                        av0[0:65, vs:SC],
                        lhsT=vnat[kb][:, 0:65],
                        rhs=pt[:, vs:SC],
                        start=(kb == 0), stop=(kb == nkb - 1),
                    )
                    nc.tensor.matmul(
                        av1[:, vs:SC],
                        lhsT=vnat[kb][:, 64:192],
                        rhs=pt[:, SC + vs:2 * SC],
                        start=(kb == 0), stop=(kb == nkb - 1),
                    )

                # software pipeline: AV(kb-3) is emitted after scores(kb),
                # giving each exp ~two iterations of cover
                pending = []
                for kb in range(nkb):
                    vs = max(0, (kb - 4 * c) * P)  # first valid col in chunk
                    st = psum.tile([P, 2 * SC], F32, tag="st", bufs=2, name="st")
                    nc.tensor.matmul(
                        st[:, vs:SC],
                        lhsT=qkrot[4][0:64, kb * P:(kb + 1) * P],
                        rhs=qkrot[hp][0:64, q0 + vs:q0 + SC],
                        start=True, stop=True,
                    )
                    nc.tensor.matmul(
                        st[:, SC + vs:2 * SC],
                        lhsT=qkrot[4][64:128, kb * P:(kb + 1) * P],
                        rhs=qkrot[hp][64:128, q0 + vs:q0 + SC],
                        start=True, stop=True,
                    )
                    if len(pending) >= 3:
                        emit_av(*pending.pop(0))
                    pt = ptp.tile([P, 2 * SC], BF16, tag="pt", name="pt")
                    # one exp over [vs:1024]: the dead span [SC:SC+vs] is
                    # unwritten PSUM (may exp to junk; never read)
                    nc.scalar.activation(
                        pt[:, vs:2 * SC], st[:, vs:2 * SC], EXP, scale=0.125
                    )
                    if kb - 4 * c >= 0:  # diagonal block: mask triangle
                        nc.vector.tensor_mul(
                            pt[:, vs:vs + P], pt[:, vs:vs + P], mask_sb
                        )
                        nc.gpsimd.tensor_mul(
                            pt[:, SC + vs:SC + vs + P],
                            pt[:, SC + vs:SC + vs + P], mask_sb
                        )
                    pending.append((kb, pt, vs))
                    it += 1
                    want = (it * n_units) // total_iters
                    while spent < want:
                        try:
                            next(filler)
                            spent += 1
                        except StopIteration:
                            spent = want
                            break
                for pp in pending:
                    emit_av(*pp)
                    try:
                        next(filler)
                        spent += 1
                    except StopIteration:
                        pass

                # ---- normalize: evict avs to SBUF first (frees the av
                # PSUM banks for the next pg's AV accumulation), then
                # den->recip->broadcast->mul chains off SBUF. B-chain
                # (den1 at partition 0, no DMA) is emitted first so its
                # ops run while dA's DMA is in flight.
                av0e = nrm.tile([65, SC], F32, tag="av0e", bufs=1, name="av0e")
                nc.vector.tensor_copy(av0e, av0[0:65, :])
                av1e = nrm.tile([P, SC], F32, tag="av1e", bufs=1, name="av1e")
                nc.vector.tensor_copy(av1e, av1)
                dA = nrm.tile([1, SC], F32, tag="dA", bufs=1, name="dA")
                nc.sync.dma_start(out=dA, in_=av0e[64:65, :])
                rB = nrm.tile([1, SC], F32, tag="rB", bufs=1, name="rB")
                nc.vector.reciprocal_approx_fast(rB, av1e[0:1, :])
                rbB = nrm.tile([P, SC], F32, tag="rbB", bufs=1, name="rbB")
                nc.gpsimd.partition_broadcast(rbB, rB)
                nc.gpsimd.tensor_mul(
                    attnT[hp][64:128, q0:q0 + SC], av1e[64:128, :], rbB[64:128, :]
                )
                rA = nrm.tile([1, SC], F32, tag="rA", bufs=1, name="rA")
                nc.vector.reciprocal_approx_fast(rA, dA)
                rbA = nrm.tile([64, SC], F32, tag="rbA", bufs=1, name="rbA")
                nc.gpsimd.partition_broadcast(rbA, rA)
                nc.gpsimd.tensor_mul(
                    attnT[hp][0:64, q0:q0 + SC], av0e[0:64, :], rbA
                )

            # drain remaining filler (next chunk depends on its qkrot/vnat)
            for _ in filler:
                pass
        # last chunk's o_proj tail
        for _ in gen_o_chunk(NSC - 1):
            pass

    nc.finalize()
    return nc


def _pack16(a, rows):
    """[rows*128, N] -> [128, rows*N] (row-chunk-major columns)."""
    n = a.shape[1]
    return np.ascontiguousarray(
        a.reshape(rows, P, n).transpose(1, 0, 2).reshape(P, rows * n)
    )


def prep_core_inputs(x, cos, sin, wq, wk, wv, wo, core, _shared={}):
    """Build the per-core input map (all host-side numpy)."""
    b, g = core // 4, core % 4
    S = x.shape[1]
    NHID = HID // P
    NSC = S // SC

    key = ("xTp", b, id(x))
    if key not in _shared:
        _shared.clear() if len(_shared) > 8 else None
        # [128, NHID*S] with column layout (s-chunk c, hid chunk h, s')
        xT = x[b].T.astype(NP_BF16)  # [HID, S]
        a = xT.reshape(NHID, P, NSC, SC).transpose(1, 2, 0, 3)
        _shared[key] = np.ascontiguousarray(a.reshape(P, NHID * S))
    xTp = _shared[key]

    qcols = []
    for i in range(4):
        h0, h1 = 8 * g + i, 8 * g + i + 4
        qcols.append(wq[:, h0 * D:(h0 + 1) * D])
        qcols.append(wq[:, h1 * D:(h1 + 1) * D])
    kcols = wk[:, 2 * g * D:(2 * g + 2) * D]
    vcols = wv[:, 2 * g * D:(2 * g + 2) * D]
    wqkv_c = np.concatenate(qcols + [kcols, vcols], axis=1).astype(NP_BF16)
    wqkvp = _pack16(wqkv_c, NHID)          # [128, NHID*768]
    worows = []
    for i in range(4):
        h0, h1 = 8 * g + i, 8 * g + i + 4
        worows.append(wo[h0 * D:(h0 + 1) * D, :])
        worows.append(wo[h1 * D:(h1 + 1) * D, :])
    wo_c = np.concatenate(worows, axis=0).astype(NP_BF16)
    wop = _pack16(wo_c, 4)                  # [128, 4*HID]

    cosT = np.tile(cos[:S].T, (2, 1)).astype(NP_BF16)
    sinT_h = np.concatenate([-sin[:S].T[:D // 2], sin[:S].T[D // 2:]], axis=0)
    sinT = np.tile(sinT_h, (2, 1)).astype(NP_BF16)
    trimask = np.triu(np.ones((P, P), dtype=NP_BF16))
    csm = np.ascontiguousarray(
        np.concatenate([cosT, sinT, trimask], axis=1)
    )

    return {"xTp": xTp, "wqkvp": wqkvp, "csm": csm, "wop": wop}


def kernel(x, cos, sin, wq, wk, wv, wo):
    x = np.asarray(x)
    S = x.shape[1]
    assert x.shape == (B, S, HID)
    if S not in _CACHE:
        _CACHE[S] = build_nc(S)
    nc = _CACHE[S]
    in_maps = [
        prep_core_inputs(x, np.asarray(cos), np.asarray(sin), np.asarray(wq),
                         np.asarray(wk), np.asarray(wv), np.asarray(wo), core)
        for core in range(8)
    ]
    res = run_bass_kernel_spmd(nc, in_maps, core_ids=list(range(8)))
    out = np.zeros((B, S, HID), np.float32)
    for core in range(8):
        out[core // 4] += res.results[core]["o_part"].astype(np.float32)
    return out


# revision 7
# speedup vs baseline: 1.3430x; 1.3430x over previous
"""GQA attention (RoPE + causal softmax + o_proj) on 8 Trainium2 NeuronCores.

Sharding: core = b*4 + g where b = batch (2), g = head-group (4).
Each core handles 8 query heads (global 8g..8g+7) and their 2 KV heads
(2g, 2g+1) for one batch element, producing a partial o_proj output
(contraction over its 512 of the 2048 hd dims). The host sums the 4
partials per batch element (o_part is bf16; host upcasts).

Per-core layout/schedule (all matmul operands bf16, fp32 PSUM accum):
  - Inputs are host-packed into 4 flat [128, N] tensors so the whole
    input load is 5 DMA instructions, issued from sync/scalar/vector
    queues in parallel (DMA issue costs ~0.7us of the issuing engine's
    queue REGARDLESS of size, so many small input DMAs serialize the
    sync queue and block the rope-swap DMAs behind them).
  - q^T/k^T built per 128-row chunk pairing heads (i, i+4); scores are
    computed transposed (S^T[k,q]) as two row-tiled K=64 matmuls that
    run concurrently in the PE array.
  - AV stationary vnat[kb] = [v0(0:64) | 1 | 0*63 | v1(128:192)]:
      av0 = vnat[:,0:65].T  @ pt0 -> v0 at partitions 0:64, den0 at 64
      av1 = vnat[:,64:192].T @ pt1 -> den1 at partition 0, v1 at 64:128
    so attnT rows 64:128 are written lane-aligned (no SBUF->SBUF DMA)
    and den1 feeds reciprocal/broadcast without a partition move.
  - At pg end avs are evicted to SBUF immediately (frees the 2 "av"
    PSUM banks for the next pg), then den->recip->broadcast->mul chains
    run off SBUF (muls on GPSIMD; recip/evict on DVE).
  - Schedule: proj(0) dense, then attention chunk c interleaves
    proj(c+1) + o_proj(c-1) units as PE filler (own "fil" PSUM slots)
    so ScalarE exp (the per-kb rate limiter) starts early and PE never
    drains. PSUM: st 2x2 + av 2 + fil 2 = 8 banks.
  - Engine balance: exp exclusively on ACT; PSUM reads (qraw/t1/
    evictions) on DVE; rope t2+add, diag-mask half, norm muls on GPSIMD.
"""

import numpy as np
import ml_dtypes
from contextlib import ExitStack

import concourse.mybir as mybir
from concourse import bacc
from concourse.tile import TileContext
from concourse.bass_utils import run_bass_kernel_spmd

BF16 = mybir.dt.bfloat16
F32 = mybir.dt.float32
NP_BF16 = ml_dtypes.bfloat16

HID = 2048
D = 64
H = 32           # global query heads
KV = 8           # global kv heads
B = 2
P = 128
SC = 512         # q-chunk width (also matmul free dim / PSUM bank)

_CACHE = {}


def build_nc(S):
    assert S % SC == 0
    NHID = HID // P       # hid chunks (16)
    NSB = S // P          # 128-row s-blocks
    NSC = S // SC         # 512-col s-chunks
    QCH = 4               # q chunk-pairs
    EXP = mybir.ActivationFunctionType.Exp

    nc = bacc.Bacc("TRN2", target_bir_lowering=False, debug=False)
    # host-packed flat inputs (see prep_core_inputs for layouts)
    xTp = nc.dram_tensor("xTp", [P, NHID * S], BF16, kind="ExternalInput")
    wqkvp = nc.dram_tensor("wqkvp", [P, NHID * 768], BF16, kind="ExternalInput")
    csm = nc.dram_tensor("csm", [P, 2 * S + P], BF16, kind="ExternalInput")
    wop = nc.dram_tensor("wop", [P, 4 * HID], BF16, kind="ExternalInput")
    o_part = nc.dram_tensor("o_part", [S, HID], BF16, kind="ExternalOutput")

    with TileContext(nc) as tc, ExitStack() as ctx:
        res = ctx.enter_context(tc.tile_pool(name="res", bufs=1))
        rope = ctx.enter_context(tc.tile_pool(name="rope", bufs=2))
        ptp = ctx.enter_context(tc.tile_pool(name="ptp", bufs=5))
        nrm = ctx.enter_context(tc.tile_pool(name="nrm", bufs=2))
        obp = ctx.enter_context(tc.tile_pool(name="obp", bufs=2))
        psum = ctx.enter_context(tc.tile_pool(name="psum", bufs=1, space="PSUM"))

        # ---- input DMA: 5 big transfers on 3 independent queues ----
        xtall = res.tile([P, NHID * S], BF16, tag="xtall", name="xtall")
        nc.sync.dma_start(out=xtall[:, 0:NHID * SC], in_=xTp[:, 0:NHID * SC])
        wqall = res.tile([P, NHID * 768], BF16, tag="wqall", name="wqall")
        nc.scalar.dma_start(out=wqall, in_=wqkvp[:, :])
        csm_t = res.tile([P, 2 * S + P], BF16, tag="csm", name="csm")
        nc.scalar.dma_start(out=csm_t, in_=csm[:, :])
        nc.scalar.dma_start(
            out=xtall[:, NHID * SC:], in_=xTp[:, NHID * SC:]
        )
        woall = res.tile([P, 4 * HID], BF16, tag="woall", name="woall")
        nc.scalar.dma_start(out=woall, in_=wop[:, :])

        # packed-layout views
        def xtv(h, s):      # x^T [hid chunk h, s-chunk s] -> [128, 512]
            base = (s * NHID + h) * SC
            return xtall[:, base:base + SC]

        def xtvb(h, sb):    # x^T [hid chunk h, s-block sb] -> [128, 128]
            base = ((sb // 4) * NHID + h) * SC + (sb % 4) * P
            return xtall[:, base:base + P]

        def wqv(h, m):      # wqkv [hid chunk h, col chunk m] (m=4 -> k)
            return wqall[:, h * 768 + m * P: h * 768 + (m + 1) * P]

        def wvv(h):         # wv [hid chunk h] -> [128, 128]
            return wqall[:, h * 768 + 640: h * 768 + 768]

        cos_sb = csm_t[:, 0:S]
        sin_sb = csm_t[:, S:2 * S]
        mask_sb = csm_t[:, 2 * S:2 * S + P]

        def wov(i, n):      # wo [hd chunk i, hid cols n*SC..] -> [128, 512]
            return woall[:, i * HID + n * SC: i * HID + (n + 1) * SC]

        # chunks 0-3: q head pairs (i, i+4); chunk 4: k (kv0 rows 0-63, kv1 64-127)
        qkrot = []
        for m in range(5):
            t = res.tile([P, S], BF16, tag=f"qkrot{m}", name=f"qkrot{m}")
            qkrot.append(t)
        # v tiles [128, 192]: [v0(0:64) | 1 | 0*63 | v1(128:192)]
        vnat = [res.tile([P, 192], BF16, tag=f"vnat{sb}", name=f"vnat{sb}")
                for sb in range(NSB)]
        # attnT is a 2-chunk column ring: chunk c lives at cols (c%2)*SC
        attnT = []
        for i in range(QCH):
            t = res.tile([P, 2 * SC], BF16, tag=f"attnT{i}", name=f"attnT{i}")
            attnT.append(t)

        def gen_o_chunk(c):
            for qb in range(4 * c, 4 * c + 4):
                ob = obp.tile([P, HID], BF16, tag="ob", name="ob")
                for n in range(HID // SC):
                    po = psum.tile([P, SC], F32, tag="fil", bufs=2, name="po")
                    for i in range(QCH):
                        nc.tensor.matmul(
                            po,
                            lhsT=attnT[i][:, (c % 2) * SC + (qb - 4 * c) * P:
                                          (c % 2) * SC + (qb - 4 * c + 1) * P],
                            rhs=wov(i, n),
                            start=(i == 0),
                            stop=(i == QCH - 1),
                        )
                    nc.vector.tensor_copy(ob[:, n * SC:(n + 1) * SC], po)
                    yield
                nc.sync.dma_start(out=o_part[qb * P:(qb + 1) * P, :], in_=ob)

        def gen_proj_schunk(s):
            """Emit s-chunk s projections + RoPE + v as units (yields).

            Matmul groups stay consecutive; eviction units only follow
            completed groups. Rope eviction split: qraw/t1 on DVE (the
            PSUM readers), t2 + final add on GPSIMD so the DVE FIFO
            never head-of-line-blocks on the qswp DMA chain."""
            sl = slice(s * SC, (s + 1) * SC)
            for m in (4, 0, 1, 2, 3):
                ps = psum.tile([P, SC], F32, tag="fil", bufs=2, name="ps_proj")
                for h0 in (0, 8):
                    for h in range(h0, h0 + 8):
                        nc.tensor.matmul(
                            ps,
                            lhsT=wqv(h, m),
                            rhs=xtv(h, s),
                            start=(h == 0),
                            stop=(h == NHID - 1),
                        )
                    yield
                # rotate_half operand: engines are lane-locked, so the
                # +-32-partition swap must go through DMA (SBUF->SBUF);
                # one DMA per direction with a 3-level partition pattern
                qraw = rope.tile([P, SC], BF16, tag="qraw", bufs=2, name="qraw")
                nc.vector.tensor_copy(qraw, ps)
                qswp = rope.tile([P, SC], BF16, tag="qswp", bufs=2, name="qswp")
                for dst, src in ((0, 32), (32, 0), (64, 96), (96, 64)):
                    nc.sync.dma_start(
                        out=qswp[dst:dst + 32, :], in_=qraw[src:src + 32, :]
                    )
                t1 = rope.tile([P, SC], BF16, tag="t1", bufs=2, name="t1")
                nc.vector.tensor_mul(t1, ps, cos_sb[:, sl])
                nc.gpsimd.tensor_mul(qswp, qswp, sin_sb[:, sl])
                nc.gpsimd.tensor_add(qkrot[m][:, sl], t1, qswp)
                yield
            for sb in range(4 * s, 4 * s + 4):
                t = vnat[sb]
                nc.gpsimd.memset(t[:, 64:65], 1.0)
                nc.gpsimd.memset(t[:, 65:128], 0.0)
                pv = psum.tile([P, 128], F32, tag="fil", bufs=2, name="ps_v")
                for h in range(NHID):
                    nc.tensor.matmul(
                        pv,
                        lhsT=xtvb(h, sb),
                        rhs=wvv(h),
                        start=(h == 0),
                        stop=(h == NHID - 1),
                    )
                yield
                nc.vector.tensor_copy(t[:, 0:64], pv[:, 0:64])
                nc.vector.tensor_copy(t[:, 128:192], pv[:, 64:128])
                yield

        def interleave(*gens):
            gens = [g for g in gens if g is not None]
            i = 0
            while gens:
                g = gens[i % len(gens)]
                try:
                    next(g)
                except StopIteration:
                    gens.remove(g)
                    continue
                yield
                i += 1

        # ---- proj chunk 0 dense (nothing to overlap with yet) ----
        with nc.named_scope("projA0"):
            for _ in gen_proj_schunk(0):
                pass

        # ---- attention chunks; proj(c+1) and o(c-1) drip into the kb
        # loop as PE fill work while ScalarE streams exps ----
        for c in range(NSC):
          with nc.named_scope(f"attn_c{c}"):
            q0 = c * SC
            rs = (c % 2) * SC
            nkb = 4 * c + 4
            filler = interleave(
                gen_proj_schunk(c + 1) if c + 1 < NSC else None,
                gen_o_chunk(c - 1) if c >= 1 else None,
            )
            n_units = (23 if c + 1 < NSC else 0) + (16 if c >= 1 else 0)
            total_iters = 4 * nkb
            it = 0
            spent = 0
            for pg in (0, 1, 2, 3):
                av0 = psum.tile([P, SC], F32, tag="av", bufs=2, name="av0")
                av1 = psum.tile([P, SC], F32, tag="av", bufs=2, name="av1")
                hp = pg

                def emit_av(kb, pt, vs):
                    nc.tensor.matmul(
                        av0[0:65, vs:SC],
                        lhsT=vnat[kb][:, 0:65],
                        rhs=pt[:, vs:SC],
                        start=(kb == 0), stop=(kb == nkb - 1),
                    )
                    nc.tensor.matmul(
                        av1[:, vs:SC],
                        lhsT=vnat[kb][:, 64:192],
                        rhs=pt[:, SC + vs:2 * SC],
                        start=(kb == 0), stop=(kb == nkb - 1),
                    )

                # software pipeline: AV(kb-3) is emitted after scores(kb),
                # giving each exp ~two iterations of cover
                pending = []
                for kb in range(nkb):
                    vs = max(0, (kb - 4 * c) * P)  # first valid col in chunk
                    st = psum.tile([P, 2 * SC], F32, tag="st", bufs=2, name="st")
                    nc.tensor.matmul(
                        st[:, vs:SC],
                        lhsT=qkrot[4][0:64, kb * P:(kb + 1) * P],
                        rhs=qkrot[hp][0:64, q0 + vs:q0 + SC],
                        start=True, stop=True,
                    )
                    nc.tensor.matmul(
                        st[:, SC + vs:2 * SC],
                        lhsT=qkrot[4][64:128, kb * P:(kb + 1) * P],
                        rhs=qkrot[hp][64:128, q0 + vs:q0 + SC],
                        start=True, stop=True,
                    )
                    if len(pending) >= 3:
                        emit_av(*pending.pop(0))
                    pt = ptp.tile([P, 2 * SC], BF16, tag="pt", name="pt")
                    # one exp over [vs:1024]: the dead span [SC:SC+vs] is
                    # unwritten PSUM (may exp to junk; never read)
                    nc.scalar.activation(
                        pt[:, vs:2 * SC], st[:, vs:2 * SC], EXP, scale=0.125
                    )
                    if kb - 4 * c >= 0:  # diagonal block: mask triangle
                        nc.vector.tensor_mul(
                            pt[:, vs:vs + P], pt[:, vs:vs + P], mask_sb
                        )
                        nc.gpsimd.tensor_mul(
                            pt[:, SC + vs:SC + vs + P],
                            pt[:, SC + vs:SC + vs + P], mask_sb
                        )
                    pending.append((kb, pt, vs))
                    it += 1
                    want = (it * n_units) // total_iters
                    while spent < want:
                        try:
                            next(filler)
                            spent += 1
                        except StopIteration:
                            spent = want
                            break
                for pp in pending:
                    emit_av(*pp)
                    try:
                        next(filler)
                        spent += 1
                    except StopIteration:
                        pass

                # ---- normalize: evict avs to SBUF first (frees the av
                # PSUM banks for the next pg's AV accumulation), then
                # den->recip->broadcast->mul chains off SBUF. B-chain
                # (den1 at partition 0, no DMA) is emitted first so its
                # ops run while dA's DMA is in flight.
                av0e = nrm.tile([65, SC], F32, tag="av0e", bufs=2, name="av0e")
                nc.vector.tensor_copy(av0e, av0[0:65, :])
                av1e = nrm.tile([P, SC], F32, tag="av1e", bufs=2, name="av1e")
                nc.vector.tensor_copy(av1e, av1)
                dA = nrm.tile([1, SC], F32, tag="dA", bufs=2, name="dA")
                nc.sync.dma_start(out=dA, in_=av0e[64:65, :])
                rB = nrm.tile([1, SC], F32, tag="rB", bufs=2, name="rB")
                nc.vector.reciprocal_approx_fast(rB, av1e[0:1, :])
                rbB = nrm.tile([P, SC], F32, tag="rbB", bufs=2, name="rbB")
                nc.gpsimd.partition_broadcast(rbB, rB)
                nc.vector.tensor_mul(
                    attnT[hp][64:128, rs:rs + SC], av1e[64:128, :], rbB[64:128, :]
                )
                rA = nrm.tile([1, SC], F32, tag="rA", bufs=2, name="rA")
                nc.vector.reciprocal_approx_fast(rA, dA)
                rbA = nrm.tile([64, SC], F32, tag="rbA", bufs=2, name="rbA")
                nc.gpsimd.partition_broadcast(rbA, rA)
                nc.vector.tensor_mul(
                    attnT[hp][0:64, rs:rs + SC], av0e[0:64, :], rbA
                )

            # drain remaining filler (next chunk depends on its qkrot/vnat)
            for _ in filler:
                pass
        # last chunk's o_proj tail
        for _ in gen_o_chunk(NSC - 1):
            pass

    nc.finalize()
    return nc


def _pack16(a, rows):
    """[rows*128, N] -> [128, rows*N] (row-chunk-major columns)."""
    n = a.shape[1]
    return np.ascontiguousarray(
        a.reshape(rows, P, n).transpose(1, 0, 2).reshape(P, rows * n)
    )


def prep_core_inputs(x, cos, sin, wq, wk, wv, wo, core, _shared={}):
    """Build the per-core input map (all host-side numpy)."""
    b, g = core // 4, core % 4
    S = x.shape[1]
    NHID = HID // P
    NSC = S // SC

    key = ("xTp", b, id(x))
    if key not in _shared:
        _shared.clear() if len(_shared) > 8 else None
        # [128, NHID*S] with column layout (s-chunk c, hid chunk h, s')
        xT = x[b].T.astype(NP_BF16)  # [HID, S]
        a = xT.reshape(NHID, P, NSC, SC).transpose(1, 2, 0, 3)
        _shared[key] = np.ascontiguousarray(a.reshape(P, NHID * S))
    xTp = _shared[key]

    qcols = []
    for i in range(4):
        h0, h1 = 8 * g + i, 8 * g + i + 4
        qcols.append(wq[:, h0 * D:(h0 + 1) * D])
        qcols.append(wq[:, h1 * D:(h1 + 1) * D])
    kcols = wk[:, 2 * g * D:(2 * g + 2) * D]
    vcols = wv[:, 2 * g * D:(2 * g + 2) * D]
    wqkv_c = np.concatenate(qcols + [kcols, vcols], axis=1).astype(NP_BF16)
    wqkvp = _pack16(wqkv_c, NHID)          # [128, NHID*768]
    worows = []
    for i in range(4):
        h0, h1 = 8 * g + i, 8 * g + i + 4
        worows.append(wo[h0 * D:(h0 + 1) * D, :])
        worows.append(wo[h1 * D:(h1 + 1) * D, :])
    wo_c = np.concatenate(worows, axis=0).astype(NP_BF16)
    wop = _pack16(wo_c, 4)                  # [128, 4*HID]

    cosT = np.tile(cos[:S].T, (2, 1)).astype(NP_BF16)
    sinT_h = np.concatenate([-sin[:S].T[:D // 2], sin[:S].T[D // 2:]], axis=0)
    sinT = np.tile(sinT_h, (2, 1)).astype(NP_BF16)
    trimask = np.triu(np.ones((P, P), dtype=NP_BF16))
    csm = np.ascontiguousarray(
        np.concatenate([cosT, sinT, trimask], axis=1)
    )

    return {"xTp": xTp, "wqkvp": wqkvp, "csm": csm, "wop": wop}


def kernel(x, cos, sin, wq, wk, wv, wo):
    x = np.asarray(x)
    S = x.shape[1]
    assert x.shape == (B, S, HID)
    if S not in _CACHE:
        _CACHE[S] = build_nc(S)
    nc = _CACHE[S]
    in_maps = [
        prep_core_inputs(x, np.asarray(cos), np.asarray(sin), np.asarray(wq),
                         np.asarray(wk), np.asarray(wv), np.asarray(wo), core)
        for core in range(8)
    ]
    res = run_bass_kernel_spmd(nc, in_maps, core_ids=list(range(8)))
    out = np.zeros((B, S, HID), np.float32)
    for core in range(8):
        out[core // 4] += res.results[core]["o_part"].astype(np.float32)
    return out


# revision 9
# speedup vs baseline: 1.6332x; 1.2161x over previous
"""GQA attention (RoPE + causal softmax + o_proj) on 8 Trainium2 NeuronCores.

Sharding: core = b*4 + g where b = batch (2), g = head-group (4).
Each core handles 8 query heads (global 8g..8g+7) and their 2 KV heads
(2g, 2g+1) for one batch element, producing a partial o_proj output
(contraction over its 512 of the 2048 hd dims). The host sums the 4
partials per batch element (o_part is bf16; host upcasts).

Per-core layout/schedule (all matmul operands bf16, fp32 PSUM accum):
  - Inputs are host-packed into 4 flat [128, N] tensors so the whole
    input load is 5 DMA instructions, issued from sync/scalar/vector
    queues in parallel (DMA issue costs ~0.7us of the issuing engine's
    queue REGARDLESS of size, so many small input DMAs serialize the
    sync queue and block the rope-swap DMAs behind them).
  - q^T/k^T built per 128-row chunk pairing heads (i, i+4); scores are
    computed transposed (S^T[k,q]) as two row-tiled K=64 matmuls that
    run concurrently in the PE array.
  - AV stationary vnat[kb] = [v0(0:64) | 1 | 0*63 | v1(128:192)]:
      av0 = vnat[:,0:65].T  @ pt0 -> v0 at partitions 0:64, den0 at 64
      av1 = vnat[:,64:192].T @ pt1 -> den1 at partition 0, v1 at 64:128
    so attnT rows 64:128 are written lane-aligned (no SBUF->SBUF DMA)
    and den1 feeds reciprocal/broadcast without a partition move.
  - At pg end avs are evicted to SBUF immediately (frees the 2 "av"
    PSUM banks for the next pg), then den->recip->broadcast->mul chains
    run off SBUF (muls on GPSIMD; recip/evict on DVE).
  - Schedule: proj(0) dense, then attention chunk c interleaves
    proj(c+1) + o_proj(c-1) units as PE filler (own "fil" PSUM slots)
    so ScalarE exp (the per-kb rate limiter) starts early and PE never
    drains. PSUM: st 2x2 + av 2 + fil 2 = 8 banks.
  - Engine balance: exp exclusively on ACT; PSUM reads (qraw/t1/
    evictions) on DVE; rope t2+add, diag-mask half, norm muls on GPSIMD.
"""

import numpy as np
import ml_dtypes
from contextlib import ExitStack

import concourse.mybir as mybir
from concourse import bacc
from concourse.tile import TileContext
from concourse.bass_utils import run_bass_kernel_spmd

BF16 = mybir.dt.bfloat16
F32 = mybir.dt.float32
NP_BF16 = ml_dtypes.bfloat16

HID = 2048
D = 64
H = 32           # global query heads
KV = 8           # global kv heads
B = 2
P = 128
SC = 512         # q-chunk width (also matmul free dim / PSUM bank)

_CACHE = {}


def build_nc(S):
    assert S % SC == 0
    NHID = HID // P       # hid chunks (16)
    NSB = S // P          # 128-row s-blocks
    NSC = S // SC         # 512-col s-chunks
    QCH = 4               # q chunk-pairs
    EXP = mybir.ActivationFunctionType.Exp

    nc = bacc.Bacc("TRN2", target_bir_lowering=False, debug=False)
    # host-packed flat inputs (see prep_core_inputs for layouts)
    xTp = nc.dram_tensor("xTp", [P, NHID * S], BF16, kind="ExternalInput")
    wqkvp = nc.dram_tensor("wqkvp", [P, NHID * 768], BF16, kind="ExternalInput")
    csm = nc.dram_tensor("csm", [P, 2 * S + P], BF16, kind="ExternalInput")
    wop = nc.dram_tensor("wop", [P, 4 * HID], BF16, kind="ExternalInput")
    o_part = nc.dram_tensor("o_part", [S, HID], BF16, kind="ExternalOutput")

    with TileContext(nc) as tc, ExitStack() as ctx:
        res = ctx.enter_context(tc.tile_pool(name="res", bufs=1))
        rope = ctx.enter_context(tc.tile_pool(name="rope", bufs=2))
        ptp = ctx.enter_context(tc.tile_pool(name="ptp", bufs=5))
        nrm = ctx.enter_context(tc.tile_pool(name="nrm", bufs=2))
        obp = ctx.enter_context(tc.tile_pool(name="obp", bufs=2))
        psum = ctx.enter_context(tc.tile_pool(name="psum", bufs=1, space="PSUM"))

        # ---- input DMA: 5 big transfers on 3 independent queues ----
        xtall = res.tile([P, NHID * S], BF16, tag="xtall", name="xtall")
        wqall = res.tile([P, NHID * 768], BF16, tag="wqall", name="wqall")
        for q in range(4):      # progressive: h-quarter q arrives early
            wsl = slice(q * 4 * 768, (q + 1) * 4 * 768)
            nc.scalar.dma_start(out=wqall[:, wsl], in_=wqkvp[:, wsl])
            xsl = slice(q * 4 * SC, (q + 1) * 4 * SC)
            nc.sync.dma_start(out=xtall[:, xsl], in_=xTp[:, xsl])
        csm_t = res.tile([P, 2 * S + P], BF16, tag="csm", name="csm")
        nc.scalar.dma_start(out=csm_t, in_=csm[:, :])
        nc.scalar.dma_start(
            out=xtall[:, NHID * SC:], in_=xTp[:, NHID * SC:]
        )
        woall = res.tile([P, 4 * HID], BF16, tag="woall", name="woall")
        nc.scalar.dma_start(out=woall, in_=wop[:, :])

        # packed-layout views
        def xtv(h, s):      # x^T [hid chunk h, s-chunk s] -> [128, 512]
            base = (s * NHID + h) * SC
            return xtall[:, base:base + SC]

        def xtvb(h, sb):    # x^T [hid chunk h, s-block sb] -> [128, 128]
            base = ((sb // 4) * NHID + h) * SC + (sb % 4) * P
            return xtall[:, base:base + P]

        def wqv(h, m):      # wqkv [hid chunk h, col chunk m] (m=4 -> k)
            return wqall[:, h * 768 + m * P: h * 768 + (m + 1) * P]

        def wvv(h):         # wv [hid chunk h] -> [128, 128]
            return wqall[:, h * 768 + 640: h * 768 + 768]

        cos_sb = csm_t[:, 0:S]
        sin_sb = csm_t[:, S:2 * S]
        mask_sb = csm_t[:, 2 * S:2 * S + P]

        def wov(i, n):      # wo [hd chunk i, hid cols n*SC..] -> [128, 512]
            return woall[:, i * HID + n * SC: i * HID + (n + 1) * SC]

        # chunks 0-3: q head pairs (i, i+4); chunk 4: k (kv0 rows 0-63, kv1 64-127)
        qkrot = []
        for m in range(5):
            t = res.tile([P, S], BF16, tag=f"qkrot{m}", name=f"qkrot{m}")
            qkrot.append(t)
        # v tiles [128, 192]: [v0(0:64) | 1 | 0*63 | v1(128:192)]
        vnat = [res.tile([P, 192], BF16, tag=f"vnat{sb}", name=f"vnat{sb}")
                for sb in range(NSB)]
        # attnT is a 2-chunk column ring: chunk c lives at cols (c%2)*SC
        attnT = []
        for i in range(QCH):
            t = res.tile([P, 2 * SC], BF16, tag=f"attnT{i}", name=f"attnT{i}")
            attnT.append(t)

        def gen_o_chunk(c):
            for qb in range(4 * c, 4 * c + 4):
                ob = obp.tile([P, HID], BF16, tag="ob", name="ob")
                for n in range(HID // SC):
                    po = psum.tile([P, SC], F32, tag="fil", bufs=2, name="po")
                    for i in range(QCH):
                        nc.tensor.matmul(
                            po,
                            lhsT=attnT[i][:, (c % 2) * SC + (qb - 4 * c) * P:
                                          (c % 2) * SC + (qb - 4 * c + 1) * P],
                            rhs=wov(i, n),
                            start=(i == 0),
                            stop=(i == QCH - 1),
                        )
                    nc.vector.tensor_copy(ob[:, n * SC:(n + 1) * SC], po)
                    yield
                nc.sync.dma_start(out=o_part[qb * P:(qb + 1) * P, :], in_=ob)

        def gen_proj_schunk(s):
            """Emit s-chunk s projections + RoPE + v as units (yields).

            Matmul groups stay consecutive; eviction units only follow
            completed groups. Rope eviction split: qraw/t1 on DVE (the
            PSUM readers), t2 + final add on GPSIMD so the DVE FIFO
            never head-of-line-blocks on the qswp DMA chain."""
            sl = slice(s * SC, (s + 1) * SC)
            for m in (4, 0, 1, 2, 3):
                ps = psum.tile([P, SC], F32, tag="fil", bufs=2, name="ps_proj")
                for h0 in (0, 8):
                    for h in range(h0, h0 + 8):
                        nc.tensor.matmul(
                            ps,
                            lhsT=wqv(h, m),
                            rhs=xtv(h, s),
                            start=(h == 0),
                            stop=(h == NHID - 1),
                        )
                    yield
                # rotate_half operand: engines are lane-locked, so the
                # +-32-partition swap must go through DMA (SBUF->SBUF);
                # one DMA per direction with a 3-level partition pattern
                qraw = rope.tile([P, SC], BF16, tag="qraw", bufs=2, name="qraw")
                nc.vector.tensor_copy(qraw, ps)
                qswp = rope.tile([P, SC], BF16, tag="qswp", bufs=2, name="qswp")
                for dst, src in ((0, 32), (32, 0), (64, 96), (96, 64)):
                    nc.sync.dma_start(
                        out=qswp[dst:dst + 32, :], in_=qraw[src:src + 32, :]
                    )
                t1 = rope.tile([P, SC], BF16, tag="t1", bufs=2, name="t1")
                nc.vector.tensor_mul(t1, ps, cos_sb[:, sl])
                nc.gpsimd.tensor_mul(qswp, qswp, sin_sb[:, sl])
                nc.gpsimd.tensor_add(qkrot[m][:, sl], t1, qswp)
                yield
            for sb in range(4 * s, 4 * s + 4):
                t = vnat[sb]
                nc.gpsimd.memset(t[:, 64:65], 1.0)
                nc.gpsimd.memset(t[:, 65:128], 0.0)
                pv = psum.tile([P, 128], F32, tag="fil", bufs=2, name="ps_v")
                for h in range(NHID):
                    nc.tensor.matmul(
                        pv,
                        lhsT=xtvb(h, sb),
                        rhs=wvv(h),
                        start=(h == 0),
                        stop=(h == NHID - 1),
                    )
                yield
                nc.vector.tensor_copy(t[:, 0:64], pv[:, 0:64])
                nc.vector.tensor_copy(t[:, 128:192], pv[:, 64:128])
                yield

        def interleave(*gens):
            gens = [g for g in gens if g is not None]
            i = 0
            while gens:
                g = gens[i % len(gens)]
                try:
                    next(g)
                except StopIteration:
                    gens.remove(g)
                    continue
                yield
                i += 1

        # ---- proj chunk 0 dense (nothing to overlap with yet) ----
        with nc.named_scope("projA0"):
            for _ in gen_proj_schunk(0):
                pass

        # ---- attention chunks; proj(c+1) and o(c-1) drip into the kb
        # loop as PE fill work while ScalarE streams exps ----
        for c in range(NSC):
          with nc.named_scope(f"attn_c{c}"):
            q0 = c * SC
            rs = (c % 2) * SC
            nkb = 4 * c + 4
            filler = interleave(
                gen_proj_schunk(c + 1) if c + 1 < NSC else None,
                gen_o_chunk(c - 1) if c >= 1 else None,
            )
            n_units = (23 if c + 1 < NSC else 0) + (16 if c >= 1 else 0)
            total_iters = 4 * nkb
            it = 0
            spent = 0
            for pg in (0, 1, 2, 3):
                av0 = psum.tile([P, SC], F32, tag="av", bufs=2, name="av0")
                av1 = psum.tile([P, SC], F32, tag="av", bufs=2, name="av1")
                hp = pg

                def emit_av(kb, pt, vs):
                    nc.tensor.matmul(
                        av0[0:65, vs:SC],
                        lhsT=vnat[kb][:, 0:65],
                        rhs=pt[:, vs:SC],
                        start=(kb == 0), stop=(kb == nkb - 1),
                    )
                    nc.tensor.matmul(
                        av1[:, vs:SC],
                        lhsT=vnat[kb][:, 64:192],
                        rhs=pt[:, SC + vs:2 * SC],
                        start=(kb == 0), stop=(kb == nkb - 1),
                    )

                # software pipeline: AV(kb-3) is emitted after scores(kb),
                # giving each exp ~two iterations of cover
                pending = []
                for kb in range(nkb):
                    vs = max(0, (kb - 4 * c) * P)  # first valid col in chunk
                    st = psum.tile([P, 2 * SC], F32, tag="st", bufs=2, name="st")
                    nc.tensor.matmul(
                        st[:, vs:SC],
                        lhsT=qkrot[4][0:64, kb * P:(kb + 1) * P],
                        rhs=qkrot[hp][0:64, q0 + vs:q0 + SC],
                        start=True, stop=True,
                    )
                    nc.tensor.matmul(
                        st[:, SC + vs:2 * SC],
                        lhsT=qkrot[4][64:128, kb * P:(kb + 1) * P],
                        rhs=qkrot[hp][64:128, q0 + vs:q0 + SC],
                        start=True, stop=True,
                    )
                    if len(pending) >= 3:
                        emit_av(*pending.pop(0))
                    pt = ptp.tile([P, 2 * SC], BF16, tag="pt", name="pt")
                    # one exp over [vs:1024]: the dead span [SC:SC+vs] is
                    # unwritten PSUM (may exp to junk; never read)
                    nc.scalar.activation(
                        pt[:, vs:2 * SC], st[:, vs:2 * SC], EXP, scale=0.125
                    )
                    if kb - 4 * c >= 0:  # diagonal block: mask triangle
                        nc.vector.tensor_mul(
                            pt[:, vs:vs + P], pt[:, vs:vs + P], mask_sb
                        )
                        nc.gpsimd.tensor_mul(
                            pt[:, SC + vs:SC + vs + P],
                            pt[:, SC + vs:SC + vs + P], mask_sb
                        )
                    pending.append((kb, pt, vs))
                    it += 1
                    want = (it * n_units) // total_iters
                    while spent < want:
                        try:
                            next(filler)
                            spent += 1
                        except StopIteration:
                            spent = want
                            break
                for pp in pending:
                    emit_av(*pp)
                    try:
                        next(filler)
                        spent += 1
                    except StopIteration:
                        pass

                # ---- normalize: evict avs to SBUF first (frees the av
                # PSUM banks for the next pg's AV accumulation), then
                # den->recip->broadcast->mul chains off SBUF. B-chain
                # (den1 at partition 0, no DMA) is emitted first so its
                # ops run while dA's DMA is in flight.
                av0e = nrm.tile([65, SC], F32, tag="av0e", bufs=2, name="av0e")
                nc.vector.tensor_copy(av0e, av0[0:65, :])
                av1e = nrm.tile([P, SC], F32, tag="av1e", bufs=2, name="av1e")
                nc.vector.tensor_copy(av1e, av1)
                # DMA-broadcast both dens (0-stride src partition dim):
                # den0 -> dn[0:64], den1 -> dn[64:128]; one reciprocal
                # covers both heads; muls are lane-aligned with attnT.
                dn = nrm.tile([P, SC], F32, tag="dn", bufs=2, name="dn")
                nc.sync.dma_start(
                    out=dn[0:64, :],
                    in_=av0e[64:65, None, :].broadcast_to([1, 64, SC]),
                )
                nc.sync.dma_start(
                    out=dn[64:128, :],
                    in_=av1e[0:1, None, :].broadcast_to([1, 64, SC]),
                )
                rc = nrm.tile([P, SC], F32, tag="rc", bufs=2, name="rc")
                nc.vector.reciprocal_approx_fast(rc, dn)
                nc.vector.tensor_mul(
                    attnT[hp][0:64, rs:rs + SC], av0e[0:64, :], rc[0:64, :]
                )
                nc.vector.tensor_mul(
                    attnT[hp][64:128, rs:rs + SC], av1e[64:128, :], rc[64:128, :]
                )

            # drain remaining filler (next chunk depends on its qkrot/vnat)
            for _ in filler:
                pass
        # last chunk's o_proj tail
        for _ in gen_o_chunk(NSC - 1):
            pass

    nc.finalize()
    return nc


def _pack16(a, rows):
    """[rows*128, N] -> [128, rows*N] (row-chunk-major columns)."""
    n = a.shape[1]
    return np.ascontiguousarray(
        a.reshape(rows, P, n).transpose(1, 0, 2).reshape(P, rows * n)
    )


def prep_core_inputs(x, cos, sin, wq, wk, wv, wo, core, _shared={}):
    """Build the per-core input map (all host-side numpy)."""
    b, g = core // 4, core % 4
    S = x.shape[1]
    NHID = HID // P
    NSC = S // SC

    key = ("xTp", b, id(x))
    if key not in _shared:
        _shared.clear() if len(_shared) > 8 else None
        # [128, NHID*S] with column layout (s-chunk c, hid chunk h, s')
        xT = x[b].T.astype(NP_BF16)  # [HID, S]
        a = xT.reshape(NHID, P, NSC, SC).transpose(1, 2, 0, 3)
        _shared[key] = np.ascontiguousarray(a.reshape(P, NHID * S))
    xTp = _shared[key]

    qcols = []
    for i in range(4):
        h0, h1 = 8 * g + i, 8 * g + i + 4
        qcols.append(wq[:, h0 * D:(h0 + 1) * D])
        qcols.append(wq[:, h1 * D:(h1 + 1) * D])
    kcols = wk[:, 2 * g * D:(2 * g + 2) * D]
    vcols = wv[:, 2 * g * D:(2 * g + 2) * D]
    wqkv_c = np.concatenate(qcols + [kcols, vcols], axis=1).astype(NP_BF16)
    wqkvp = _pack16(wqkv_c, NHID)          # [128, NHID*768]
    worows = []
    for i in range(4):
        h0, h1 = 8 * g + i, 8 * g + i + 4
        worows.append(wo[h0 * D:(h0 + 1) * D, :])
        worows.append(wo[h1 * D:(h1 + 1) * D, :])
    wo_c = np.concatenate(worows, axis=0).astype(NP_BF16)
    wop = _pack16(wo_c, 4)                  # [128, 4*HID]

    cosT = np.tile(cos[:S].T, (2, 1)).astype(NP_BF16)
    sinT_h = np.concatenate([-sin[:S].T[:D // 2], sin[:S].T[D // 2:]], axis=0)
    sinT = np.tile(sinT_h, (2, 1)).astype(NP_BF16)
    trimask = np.triu(np.ones((P, P), dtype=NP_BF16))
    csm = np.ascontiguousarray(
        np.concatenate([cosT, sinT, trimask], axis=1)
    )

    return {"xTp": xTp, "wqkvp": wqkvp, "csm": csm, "wop": wop}


def kernel(x, cos, sin, wq, wk, wv, wo):
    x = np.asarray(x)
    S = x.shape[1]
    assert x.shape == (B, S, HID)
    if S not in _CACHE:
        _CACHE[S] = build_nc(S)
    nc = _CACHE[S]
    in_maps = [
        prep_core_inputs(x, np.asarray(cos), np.asarray(sin), np.asarray(wq),
                         np.asarray(wk), np.asarray(wv), np.asarray(wo), core)
        for core in range(8)
    ]
    res = run_bass_kernel_spmd(nc, in_maps, core_ids=list(range(8)))
    out = np.zeros((B, S, HID), np.float32)
    for core in range(8):
        out[core // 4] += res.results[core]["o_part"].astype(np.float32)
    return out


# revision 12
# speedup vs baseline: 1.6513x; 1.0110x over previous
"""GQA attention (RoPE + causal softmax + o_proj) on 8 Trainium2 NeuronCores.

Sharding: core = b*4 + g where b = batch (2), g = head-group (4).
Each core handles 8 query heads (global 8g..8g+7) and their 2 KV heads
(2g, 2g+1) for one batch element, producing a partial o_proj output
(contraction over its 512 of the 2048 hd dims). The host sums the 4
partials per batch element (o_part is bf16; host upcasts).

Per-core layout/schedule (all matmul operands bf16, fp32 PSUM accum):
  - Inputs are host-packed into 4 flat [128, N] tensors so the whole
    input load is 5 DMA instructions, issued from sync/scalar/vector
    queues in parallel (DMA issue costs ~0.7us of the issuing engine's
    queue REGARDLESS of size, so many small input DMAs serialize the
    sync queue and block the rope-swap DMAs behind them).
  - q^T/k^T built per 128-row chunk pairing heads (i, i+4); scores are
    computed transposed (S^T[k,q]) as two row-tiled K=64 matmuls that
    run concurrently in the PE array.
  - AV stationary vnat[kb] = [v0(0:64) | 1 | 0*63 | v1(128:192)]:
      av0 = vnat[:,0:65].T  @ pt0 -> v0 at partitions 0:64, den0 at 64
      av1 = vnat[:,64:192].T @ pt1 -> den1 at partition 0, v1 at 64:128
    so attnT rows 64:128 are written lane-aligned (no SBUF->SBUF DMA)
    and den1 feeds reciprocal/broadcast without a partition move.
  - At pg end avs are evicted to SBUF immediately (frees the 2 "av"
    PSUM banks for the next pg), then den->recip->broadcast->mul chains
    run off SBUF (muls on GPSIMD; recip/evict on DVE).
  - Schedule: proj(0) dense, then attention chunk c interleaves
    proj(c+1) + o_proj(c-1) units as PE filler (own "fil" PSUM slots)
    so ScalarE exp (the per-kb rate limiter) starts early and PE never
    drains. PSUM: st 2x2 + av 2 + fil 2 = 8 banks.
  - Engine balance: exp exclusively on ACT; PSUM reads (qraw/t1/
    evictions) on DVE; rope t2+add, diag-mask half, norm muls on GPSIMD.
"""

import numpy as np
import ml_dtypes
from contextlib import ExitStack

import concourse.mybir as mybir
from concourse import bacc
from concourse.tile import TileContext
from concourse.bass_utils import run_bass_kernel_spmd

BF16 = mybir.dt.bfloat16
F32 = mybir.dt.float32
NP_BF16 = ml_dtypes.bfloat16

HID = 2048
D = 64
H = 32           # global query heads
KV = 8           # global kv heads
B = 2
P = 128
SC = 512         # q-chunk width (also matmul free dim / PSUM bank)

_CACHE = {}


def build_nc(S):
    assert S % SC == 0
    NHID = HID // P       # hid chunks (16)
    NSB = S // P          # 128-row s-blocks
    NSC = S // SC         # 512-col s-chunks
    QCH = 4               # q chunk-pairs
    EXP = mybir.ActivationFunctionType.Exp

    nc = bacc.Bacc("TRN2", target_bir_lowering=False, debug=False)
    # host-packed flat inputs (see prep_core_inputs for layouts)
    xTp = nc.dram_tensor("xTp", [P, NHID * S], BF16, kind="ExternalInput")
    wqkvp = nc.dram_tensor("wqkvp", [P, NHID * 768], BF16, kind="ExternalInput")
    csm = nc.dram_tensor("csm", [P, 2 * S + P], BF16, kind="ExternalInput")
    wop = nc.dram_tensor("wop", [P, 4 * HID], BF16, kind="ExternalInput")
    o_part = nc.dram_tensor("o_part", [S, HID], BF16, kind="ExternalOutput")

    with TileContext(nc) as tc, ExitStack() as ctx:
        res = ctx.enter_context(tc.tile_pool(name="res", bufs=1))
        rope = ctx.enter_context(tc.tile_pool(name="rope", bufs=3))
        ptp = ctx.enter_context(tc.tile_pool(name="ptp", bufs=5))
        nrm = ctx.enter_context(tc.tile_pool(name="nrm", bufs=2))
        obp = ctx.enter_context(tc.tile_pool(name="obp", bufs=2))
        psum = ctx.enter_context(tc.tile_pool(name="psum", bufs=1, space="PSUM"))

        # ---- input DMA: 5 big transfers on 3 independent queues ----
        xtall = res.tile([P, NHID * S], BF16, tag="xtall", name="xtall")
        wqall = res.tile([P, NHID * 768], BF16, tag="wqall", name="wqall")
        for q in range(4):      # progressive: h-quarter q arrives early
            wsl = slice(q * 4 * 768, (q + 1) * 4 * 768)
            nc.scalar.dma_start(out=wqall[:, wsl], in_=wqkvp[:, wsl])
            xsl = slice(q * 4 * SC, (q + 1) * 4 * SC)
            nc.sync.dma_start(out=xtall[:, xsl], in_=xTp[:, xsl])
        csm_t = res.tile([P, 2 * S + P], BF16, tag="csm", name="csm")
        nc.scalar.dma_start(out=csm_t, in_=csm[:, :])
        nc.scalar.dma_start(
            out=xtall[:, NHID * SC:], in_=xTp[:, NHID * SC:]
        )
        woall = res.tile([P, 4 * HID], BF16, tag="woall", name="woall")
        nc.scalar.dma_start(out=woall, in_=wop[:, :])

        # packed-layout views
        def xtv(h, s):      # x^T [hid chunk h, s-chunk s] -> [128, 512]
            base = (s * NHID + h) * SC
            return xtall[:, base:base + SC]

        def xtvb(h, sb):    # x^T [hid chunk h, s-block sb] -> [128, 128]
            base = ((sb // 4) * NHID + h) * SC + (sb % 4) * P
            return xtall[:, base:base + P]

        def wqv(h, m):      # wqkv [hid chunk h, col chunk m] (m=4 -> k)
            return wqall[:, h * 768 + m * P: h * 768 + (m + 1) * P]

        def wvv(h):         # wv [hid chunk h] -> [128, 128]
            return wqall[:, h * 768 + 640: h * 768 + 768]

        cos_sb = csm_t[:, 0:S]
        sin_sb = csm_t[:, S:2 * S]
        mask_sb = csm_t[:, 2 * S:2 * S + P]

        def wov(i, n):      # wo [hd chunk i, hid cols n*SC..] -> [128, 512]
            return woall[:, i * HID + n * SC: i * HID + (n + 1) * SC]

        # chunks 0-3: q head pairs (i, i+4); chunk 4: k (kv0 rows 0-63, kv1 64-127)
        qkrot = []
        for m in range(5):
            t = res.tile([P, S], BF16, tag=f"qkrot{m}", name=f"qkrot{m}")
            qkrot.append(t)
        # v tiles [128, 192]: [v0(0:64) | 1 | 0*63 | v1(128:192)]
        vnat = [res.tile([P, 192], BF16, tag=f"vnat{sb}", name=f"vnat{sb}")
                for sb in range(NSB)]
        attnT = []
        for i in range(QCH):
            t = res.tile([P, S], BF16, tag=f"attnT{i}", name=f"attnT{i}")
            attnT.append(t)

        def gen_o_chunk(c):
            for qb in range(4 * c, 4 * c + 4):
                ob = obp.tile([P, HID], BF16, tag="ob", name="ob")
                for n in range(HID // SC):
                    po = psum.tile([P, SC], F32, tag="fil", bufs=2, name="po")
                    for i in range(QCH):
                        nc.tensor.matmul(
                            po,
                            lhsT=attnT[i][:, qb * P:(qb + 1) * P],
                            rhs=wov(i, n),
                            start=(i == 0),
                            stop=(i == QCH - 1),
                        )
                    nc.vector.tensor_copy(ob[:, n * SC:(n + 1) * SC], po)
                    yield
                nc.gpsimd.dma_start(out=o_part[qb * P:(qb + 1) * P, :], in_=ob)

        def gen_proj_munits(s, ms):
            """Emit projection m-chunk units for s-chunk s (yields).

            Matmul groups stay consecutive; eviction units only follow
            completed groups. Rope eviction split: qraw/t1 on DVE (the
            PSUM readers), t2 + final add on GPSIMD so the DVE FIFO
            never head-of-line-blocks on the qswp DMA chain."""
            sl = slice(s * SC, (s + 1) * SC)
            for m in ms:
                ps = psum.tile([P, SC], F32, tag="fil", bufs=2, name="ps_proj")
                for h0 in (0, 8):
                    for h in range(h0, h0 + 8):
                        nc.tensor.matmul(
                            ps,
                            lhsT=wqv(h, m),
                            rhs=xtv(h, s),
                            start=(h == 0),
                            stop=(h == NHID - 1),
                        )
                    yield
                # rotate_half operand: engines are lane-locked, so the
                # +-32-partition swap must go through DMA (SBUF->SBUF);
                # one DMA per direction with a 3-level partition pattern
                qraw = rope.tile([P, SC], BF16, tag="qraw", bufs=3, name="qraw")
                nc.vector.tensor_copy(qraw, ps)
                qswp = rope.tile([P, SC], BF16, tag="qswp", bufs=3, name="qswp")
                for dst, src in ((0, 32), (32, 0), (64, 96), (96, 64)):
                    nc.sync.dma_start(
                        out=qswp[dst:dst + 32, :], in_=qraw[src:src + 32, :]
                    )
                t1 = rope.tile([P, SC], BF16, tag="t1", bufs=3, name="t1")
                nc.vector.tensor_mul(t1, ps, cos_sb[:, sl])
                nc.gpsimd.tensor_mul(qswp, qswp, sin_sb[:, sl])
                nc.gpsimd.tensor_add(qkrot[m][:, sl], t1, qswp)
                yield

        def gen_v_units(s):
            for sb in range(4 * s, 4 * s + 4):
                t = vnat[sb]
                nc.gpsimd.memset(t[:, 64:65], 1.0)
                nc.gpsimd.memset(t[:, 65:128], 0.0)
                pv = psum.tile([P, 128], F32, tag="fil", bufs=2, name="ps_v")
                for h in range(NHID):
                    nc.tensor.matmul(
                        pv,
                        lhsT=xtvb(h, sb),
                        rhs=wvv(h),
                        start=(h == 0),
                        stop=(h == NHID - 1),
                    )
                yield
                nc.vector.tensor_copy(t[:, 0:64], pv[:, 0:64])
                nc.vector.tensor_copy(t[:, 128:192], pv[:, 64:128])
                yield

        def gen_proj_head(s):   # pg0 deps of chunk s: k, q-pair 0, v
            yield from gen_proj_munits(s, (4, 0))
            yield from gen_v_units(s)

        def gen_proj_tail(s):   # pg1-3 deps: fillered into chunk s itself
            yield from gen_proj_munits(s, (1, 2, 3))

        def interleave(*gens):
            gens = [g for g in gens if g is not None]
            i = 0
            while gens:
                g = gens[i % len(gens)]
                try:
                    next(g)
                except StopIteration:
                    gens.remove(g)
                    continue
                yield
                i += 1

        def winterleave(specs):
            """Round-robin with weights: (gen, weight) pulls weight
            units per round. Tail-proj gets weight 2 so qkrot[m] for
            upcoming head-pairs lands ahead of their first scores."""
            active = [(g, w) for g, w in specs if g is not None]
            while active:
                nxt = []
                for g, w in active:
                    alive = True
                    for _ in range(w):
                        try:
                            next(g)
                        except StopIteration:
                            alive = False
                            break
                        yield
                    if alive:
                        nxt.append((g, w))
                active = nxt

        # ---- proj head of chunk 0 dense (nothing to overlap with) ----
        with nc.named_scope("projA0"):
            for _ in gen_proj_head(0):
                pass

        # deferred o-unit generators: chunk c may drain earlier chunks'
        # o units up to a per-chunk cap; the rest spill to later chunks
        o_gens = []
        o_caps = {0: 0, 1: 4, 2: 10, 3: 99}

        def gen_o_budget(cap):
            spent = 0
            while o_gens and spent < cap:
                try:
                    next(o_gens[0])
                except StopIteration:
                    o_gens.pop(0)
                    continue
                spent += 1
                yield

        # ---- attention chunks; proj tail(c) + head(c+1) + deferred o
        # drip into the kb loop as PE fill work while ScalarE streams
        # exps ----
        for c in range(NSC):
          with nc.named_scope(f"attn_c{c}"):
            q0 = c * SC
            nkb = 4 * c + 4
            if c >= 1:
                o_gens.append(gen_o_chunk(c - 1))
            filler = winterleave([
                (gen_proj_tail(c), 2),
                (gen_proj_head(c + 1) if c + 1 < NSC else None, 1),
                (gen_o_budget(o_caps[c]), 1),
            ])
            n_units = 9 + (10 if c + 1 < NSC else 0) + min(
                o_caps[c], 16 * max(0, min(c, 2)))
            total_iters = 4 * nkb
            it = 0
            spent = 0
            for pg in (0, 1, 2, 3):
                av0 = psum.tile([P, SC], F32, tag="av", bufs=2, name="av0")
                av1 = psum.tile([P, SC], F32, tag="av", bufs=2, name="av1")
                hp = pg

                def emit_av(kb, pt, vs):
                    nc.tensor.matmul(
                        av0[0:65, vs:SC],
                        lhsT=vnat[kb][:, 0:65],
                        rhs=pt[:, vs:SC],
                        start=(kb == 0), stop=(kb == nkb - 1),
                    )
                    nc.tensor.matmul(
                        av1[:, vs:SC],
                        lhsT=vnat[kb][:, 64:192],
                        rhs=pt[:, SC + vs:2 * SC],
                        start=(kb == 0), stop=(kb == nkb - 1),
                    )

                # software pipeline: AV(kb-3) is emitted after scores(kb),
                # giving each exp ~two iterations of cover
                pending = []
                for kb in range(nkb):
                    vs = max(0, (kb - 4 * c) * P)  # first valid col in chunk
                    st = psum.tile([P, 2 * SC], F32, tag="st", bufs=2, name="st")
                    nc.tensor.matmul(
                        st[:, vs:SC],
                        lhsT=qkrot[4][0:64, kb * P:(kb + 1) * P],
                        rhs=qkrot[hp][0:64, q0 + vs:q0 + SC],
                        start=True, stop=True,
                    )
                    nc.tensor.matmul(
                        st[:, SC + vs:2 * SC],
                        lhsT=qkrot[4][64:128, kb * P:(kb + 1) * P],
                        rhs=qkrot[hp][64:128, q0 + vs:q0 + SC],
                        start=True, stop=True,
                    )
                    if len(pending) >= 3:
                        emit_av(*pending.pop(0))
                    pt = ptp.tile([P, 2 * SC], BF16, tag="pt", name="pt")
                    # one exp over [vs:1024]: the dead span [SC:SC+vs] is
                    # unwritten PSUM (may exp to junk; never read)
                    nc.scalar.activation(
                        pt[:, vs:2 * SC], st[:, vs:2 * SC], EXP, scale=0.125
                    )
                    if kb - 4 * c >= 0:  # diagonal block: mask triangle
                        nc.vector.tensor_mul(
                            pt[:, vs:vs + P], pt[:, vs:vs + P], mask_sb
                        )
                        nc.gpsimd.tensor_mul(
                            pt[:, SC + vs:SC + vs + P],
                            pt[:, SC + vs:SC + vs + P], mask_sb
                        )
                    pending.append((kb, pt, vs))
                    it += 1
                    want = (it * n_units) // total_iters
                    while spent < want:
                        try:
                            next(filler)
                            spent += 1
                        except StopIteration:
                            spent = want
                            break
                for pp in pending:
                    emit_av(*pp)
                    try:
                        next(filler)
                        spent += 1
                    except StopIteration:
                        pass

                # ---- normalize: evict avs to SBUF first (frees the av
                # PSUM banks for the next pg's AV accumulation), then
                # den->recip->broadcast->mul chains off SBUF. B-chain
                # (den1 at partition 0, no DMA) is emitted first so its
                # ops run while dA's DMA is in flight.
                av0e = nrm.tile([65, SC], F32, tag="av0e", bufs=2, name="av0e")
                nc.vector.tensor_copy(av0e, av0[0:65, :])
                av1e = nrm.tile([P, SC], F32, tag="av1e", bufs=2, name="av1e")
                nc.vector.tensor_copy(av1e, av1)
                # DMA-broadcast both dens (0-stride src partition dim):
                # den0 -> dn[0:64], den1 -> dn[64:128]; one reciprocal
                # covers both heads; muls are lane-aligned with attnT.
                dn = nrm.tile([P, SC], F32, tag="dn", bufs=2, name="dn")
                nc.sync.dma_start(
                    out=dn[0:64, :],
                    in_=av0e[64:65, None, :].broadcast_to([1, 64, SC]),
                )
                nc.sync.dma_start(
                    out=dn[64:128, :],
                    in_=av1e[0:1, None, :].broadcast_to([1, 64, SC]),
                )
                rc = nrm.tile([P, SC], F32, tag="rc", bufs=2, name="rc")
                nc.vector.reciprocal_approx_fast(rc, dn)
                nc.vector.tensor_mul(
                    attnT[hp][0:64, q0:q0 + SC], av0e[0:64, :], rc[0:64, :]
                )
                nc.vector.tensor_mul(
                    attnT[hp][64:128, q0:q0 + SC], av1e[64:128, :], rc[64:128, :]
                )

            # drain remaining filler (next chunk depends on its qkrot/vnat)
            for _ in filler:
                pass
        # remaining deferred o units, then the last chunk's o_proj tail
        for g in o_gens:
            for _ in g:
                pass
        for _ in gen_o_chunk(NSC - 1):
            pass

    nc.finalize()
    return nc


def _pack16(a, rows):
    """[rows*128, N] -> [128, rows*N] (row-chunk-major columns)."""
    n = a.shape[1]
    return np.ascontiguousarray(
        a.reshape(rows, P, n).transpose(1, 0, 2).reshape(P, rows * n)
    )


def prep_core_inputs(x, cos, sin, wq, wk, wv, wo, core, _shared={}):
    """Build the per-core input map (all host-side numpy)."""
    b, g = core // 4, core % 4
    S = x.shape[1]
    NHID = HID // P
    NSC = S // SC

    key = ("xTp", b, id(x))
    if key not in _shared:
        _shared.clear() if len(_shared) > 8 else None
        # [128, NHID*S] with column layout (s-chunk c, hid chunk h, s')
        xT = x[b].T.astype(NP_BF16)  # [HID, S]
        a = xT.reshape(NHID, P, NSC, SC).transpose(1, 2, 0, 3)
        _shared[key] = np.ascontiguousarray(a.reshape(P, NHID * S))
    xTp = _shared[key]

    qcols = []
    for i in range(4):
        h0, h1 = 8 * g + i, 8 * g + i + 4
        qcols.append(wq[:, h0 * D:(h0 + 1) * D])
        qcols.append(wq[:, h1 * D:(h1 + 1) * D])
    kcols = wk[:, 2 * g * D:(2 * g + 2) * D]
    vcols = wv[:, 2 * g * D:(2 * g + 2) * D]
    wqkv_c = np.concatenate(qcols + [kcols, vcols], axis=1).astype(NP_BF16)
    wqkvp = _pack16(wqkv_c, NHID)          # [128, NHID*768]
    worows = []
    for i in range(4):
        h0, h1 = 8 * g + i, 8 * g + i + 4
        worows.append(wo[h0 * D:(h0 + 1) * D, :])
        worows.append(wo[h1 * D:(h1 + 1) * D, :])
    wo_c = np.concatenate(worows, axis=0).astype(NP_BF16)
    wop = _pack16(wo_c, 4)                  # [128, 4*HID]

    cosT = np.tile(cos[:S].T, (2, 1)).astype(NP_BF16)
    sinT_h = np.concatenate([-sin[:S].T[:D // 2], sin[:S].T[D // 2:]], axis=0)
    sinT = np.tile(sinT_h, (2, 1)).astype(NP_BF16)
    trimask = np.triu(np.ones((P, P), dtype=NP_BF16))
    csm = np.ascontiguousarray(
        np.concatenate([cosT, sinT, trimask], axis=1)
    )

    return {"xTp": xTp, "wqkvp": wqkvp, "csm": csm, "wop": wop}


def kernel(x, cos, sin, wq, wk, wv, wo):
    x = np.asarray(x)
    S = x.shape[1]
    assert x.shape == (B, S, HID)
    if S not in _CACHE:
        _CACHE[S] = build_nc(S)
    nc = _CACHE[S]
    in_maps = [
        prep_core_inputs(x, np.asarray(cos), np.asarray(sin), np.asarray(wq),
                         np.asarray(wk), np.asarray(wv), np.asarray(wo), core)
        for core in range(8)
    ]
    res = run_bass_kernel_spmd(nc, in_maps, core_ids=list(range(8)))
    out = np.zeros((B, S, HID), np.float32)
    for core in range(8):
        out[core // 4] += res.results[core]["o_part"].astype(np.float32)
    return out


# revision 13
# speedup vs baseline: 1.6961x; 1.0271x over previous
"""GQA attention (RoPE + causal softmax + o_proj) on 8 Trainium2 NeuronCores.

Sharding: core = b*4 + g where b = batch (2), g = head-group (4).
Each core handles 8 query heads (global 8g..8g+7) and their 2 KV heads
(2g, 2g+1) for one batch element, producing a partial o_proj output
(contraction over its 512 of the 2048 hd dims). The host sums the 4
partials per batch element (o_part is bf16; host upcasts).

Per-core layout/schedule (all matmul operands bf16, fp32 PSUM accum):
  - Inputs are host-packed into 4 flat [128, N] tensors so the whole
    input load is 5 DMA instructions, issued from sync/scalar/vector
    queues in parallel (DMA issue costs ~0.7us of the issuing engine's
    queue REGARDLESS of size, so many small input DMAs serialize the
    sync queue and block the rope-swap DMAs behind them).
  - q^T/k^T built per 128-row chunk pairing heads (i, i+4); scores are
    computed transposed (S^T[k,q]) as two row-tiled K=64 matmuls that
    run concurrently in the PE array.
  - AV stationary vnat[kb] = [v0(0:64) | 1 | 0*63 | v1(128:192)]:
      av0 = vnat[:,0:65].T  @ pt0 -> v0 at partitions 0:64, den0 at 64
      av1 = vnat[:,64:192].T @ pt1 -> den1 at partition 0, v1 at 64:128
    so attnT rows 64:128 are written lane-aligned (no SBUF->SBUF DMA)
    and den1 feeds reciprocal/broadcast without a partition move.
  - At pg end avs are evicted to SBUF immediately (frees the 2 "av"
    PSUM banks for the next pg), then den->recip->broadcast->mul chains
    run off SBUF (muls on GPSIMD; recip/evict on DVE).
  - Schedule: proj(0) dense, then attention chunk c interleaves
    proj(c+1) + o_proj(c-1) units as PE filler (own "fil" PSUM slots)
    so ScalarE exp (the per-kb rate limiter) starts early and PE never
    drains. PSUM: st 2x2 + av 2 + fil 2 = 8 banks.
  - Engine balance: exp exclusively on ACT; PSUM reads (qraw/t1/
    evictions) on DVE; rope t2+add, diag-mask half, norm muls on GPSIMD.
"""

import numpy as np
import ml_dtypes
from contextlib import ExitStack

import concourse.mybir as mybir
from concourse import bacc
from concourse.tile import TileContext
from concourse.bass_utils import run_bass_kernel_spmd

BF16 = mybir.dt.bfloat16
F32 = mybir.dt.float32
NP_BF16 = ml_dtypes.bfloat16

HID = 2048
D = 64
H = 32           # global query heads
KV = 8           # global kv heads
B = 2
P = 128
SC = 512         # q-chunk width (also matmul free dim / PSUM bank)

_CACHE = {}


def build_nc(S):
    assert S % SC == 0
    NHID = HID // P       # hid chunks (16)
    NSB = S // P          # 128-row s-blocks
    NSC = S // SC         # 512-col s-chunks
    QCH = 4               # q chunk-pairs
    EXP = mybir.ActivationFunctionType.Exp

    nc = bacc.Bacc("TRN2", target_bir_lowering=False, debug=False)
    # host-packed flat inputs (see prep_core_inputs for layouts)
    xTp = nc.dram_tensor("xTp", [P, NHID * S], BF16, kind="ExternalInput")
    wqkvp = nc.dram_tensor("wqkvp", [P, NHID * 768], BF16, kind="ExternalInput")
    csm = nc.dram_tensor("csm", [P, 2 * S + P], BF16, kind="ExternalInput")
    wop = nc.dram_tensor("wop", [P, 4 * HID], BF16, kind="ExternalInput")
    o_part = nc.dram_tensor("o_part", [S, HID], BF16, kind="ExternalOutput")

    with TileContext(nc) as tc, ExitStack() as ctx:
        res = ctx.enter_context(tc.tile_pool(name="res", bufs=1))
        rope = ctx.enter_context(tc.tile_pool(name="rope", bufs=3))
        ptp = ctx.enter_context(tc.tile_pool(name="ptp", bufs=5))
        nrm = ctx.enter_context(tc.tile_pool(name="nrm", bufs=2))
        obp = ctx.enter_context(tc.tile_pool(name="obp", bufs=2))
        psum = ctx.enter_context(tc.tile_pool(name="psum", bufs=1, space="PSUM"))

        # ---- input DMA: purpose-split queues. sync: xT chunk 0
        # (h-progressive quarters); scalar: weights m-major-progressive;
        # gpsimd: cos/sin/mask (small, needed by the first rope). The
        # rope-swap / den / o_part DMAs later share these queues by
        # latency class.
        xtall = res.tile([P, NHID * S], BF16, tag="xtall", name="xtall")
        wqall = res.tile([P, NHID * 768], BF16, tag="wqall", name="wqall")
        csm_t = res.tile([P, 2 * S + P], BF16, tag="csm", name="csm")
        nc.gpsimd.dma_start(out=csm_t, in_=csm[:, :])
        for q in range(4):      # xT c0: h-quarter q arrives early
            xsl = slice(q * 4 * SC, (q + 1) * 4 * SC)
            nc.sync.dma_start(out=xtall[:, xsl], in_=xTp[:, xsl])
        for b in range(6):      # weight blocks in consumption order
            wsl = slice(b * NHID * P, (b + 1) * NHID * P)
            nc.scalar.dma_start(out=wqall[:, wsl], in_=wqkvp[:, wsl])
        nc.scalar.dma_start(
            out=xtall[:, NHID * SC:], in_=xTp[:, NHID * SC:]
        )
        woall = res.tile([P, 4 * HID], BF16, tag="woall", name="woall")
        nc.scalar.dma_start(out=woall, in_=wop[:, :])

        # packed-layout views
        def xtv(h, s):      # x^T [hid chunk h, s-chunk s] -> [128, 512]
            base = (s * NHID + h) * SC
            return xtall[:, base:base + SC]

        def xtvb(h, sb):    # x^T [hid chunk h, s-block sb] -> [128, 128]
            base = ((sb // 4) * NHID + h) * SC + (sb % 4) * P
            return xtall[:, base:base + P]

        # wqall column layout: 6 blocks x 16 h x 128 cols, block order
        # (k, q-pair0, v, q-pair1, q-pair2, q-pair3) = consumption order
        _WBLK = {4: 0, 0: 1, 1: 3, 2: 4, 3: 5}

        def wqv(h, m):      # wqkv [hid chunk h, col chunk m] (m=4 -> k)
            base = (_WBLK[m] * NHID + h) * P
            return wqall[:, base:base + P]

        def wvv(h):         # wv [hid chunk h] -> [128, 128]
            base = (2 * NHID + h) * P
            return wqall[:, base:base + P]

        cos_sb = csm_t[:, 0:S]
        sin_sb = csm_t[:, S:2 * S]
        mask_sb = csm_t[:, 2 * S:2 * S + P]

        def wov(i, n):      # wo [hd chunk i, hid cols n*SC..] -> [128, 512]
            return woall[:, i * HID + n * SC: i * HID + (n + 1) * SC]

        # chunks 0-3: q head pairs (i, i+4); chunk 4: k (kv0 rows 0-63, kv1 64-127)
        qkrot = []
        for m in range(5):
            t = res.tile([P, S], BF16, tag=f"qkrot{m}", name=f"qkrot{m}")
            qkrot.append(t)
        # v tiles [128, 192]: [v0(0:64) | 1 | 0*63 | v1(128:192)]
        vnat = [res.tile([P, 192], BF16, tag=f"vnat{sb}", name=f"vnat{sb}")
                for sb in range(NSB)]
        attnT = []
        for i in range(QCH):
            t = res.tile([P, S], BF16, tag=f"attnT{i}", name=f"attnT{i}")
            attnT.append(t)

        def gen_o_chunk(c):
            for qb in range(4 * c, 4 * c + 4):
                ob = obp.tile([P, HID], BF16, tag="ob", name="ob")
                for n in range(HID // SC):
                    po = psum.tile([P, SC], F32, tag="fil", bufs=2, name="po")
                    for i in range(QCH):
                        nc.tensor.matmul(
                            po,
                            lhsT=attnT[i][:, qb * P:(qb + 1) * P],
                            rhs=wov(i, n),
                            start=(i == 0),
                            stop=(i == QCH - 1),
                        )
                    nc.vector.tensor_copy(ob[:, n * SC:(n + 1) * SC], po)
                    yield
                nc.gpsimd.dma_start(out=o_part[qb * P:(qb + 1) * P, :], in_=ob)

        def gen_proj_munits(s, ms):
            """Emit projection m-chunk units for s-chunk s (yields).

            Matmul groups stay consecutive; eviction units only follow
            completed groups. Rope eviction split: qraw/t1 on DVE (the
            PSUM readers), t2 + final add on GPSIMD so the DVE FIFO
            never head-of-line-blocks on the qswp DMA chain."""
            sl = slice(s * SC, (s + 1) * SC)
            for m in ms:
                ps = psum.tile([P, SC], F32, tag="fil", bufs=2, name="ps_proj")
                for h0 in (0, 8):
                    for h in range(h0, h0 + 8):
                        nc.tensor.matmul(
                            ps,
                            lhsT=wqv(h, m),
                            rhs=xtv(h, s),
                            start=(h == 0),
                            stop=(h == NHID - 1),
                        )
                    yield
                # rotate_half operand: engines are lane-locked, so the
                # +-32-partition swap must go through DMA (SBUF->SBUF);
                # one DMA per direction with a 3-level partition pattern
                qraw = rope.tile([P, SC], BF16, tag="qraw", bufs=3, name="qraw")
                nc.vector.tensor_copy(qraw, ps)
                qswp = rope.tile([P, SC], BF16, tag="qswp", bufs=3, name="qswp")
                for dst, srcp in ((0, 32), (32, 0), (64, 96), (96, 64)):
                    nc.gpsimd.dma_start(
                        out=qswp[dst:dst + 32, :], in_=qraw[srcp:srcp + 32, :]
                    )
                t1 = rope.tile([P, SC], BF16, tag="t1", bufs=3, name="t1")
                nc.vector.tensor_mul(t1, ps, cos_sb[:, sl])
                nc.gpsimd.tensor_mul(qswp, qswp, sin_sb[:, sl])
                nc.gpsimd.tensor_add(qkrot[m][:, sl], t1, qswp)
                yield

        def gen_v_units(s):
            for sb in range(4 * s, 4 * s + 4):
                t = vnat[sb]
                nc.gpsimd.memset(t[:, 64:65], 1.0)
                nc.gpsimd.memset(t[:, 65:128], 0.0)
                pv = psum.tile([P, 128], F32, tag="fil", bufs=2, name="ps_v")
                for h in range(NHID):
                    nc.tensor.matmul(
                        pv,
                        lhsT=xtvb(h, sb),
                        rhs=wvv(h),
                        start=(h == 0),
                        stop=(h == NHID - 1),
                    )
                yield
                nc.vector.tensor_copy(t[:, 0:64], pv[:, 0:64])
                nc.vector.tensor_copy(t[:, 128:192], pv[:, 64:128])
                yield

        def gen_proj_head(s):   # pg0 deps of chunk s: k, q-pair 0, v
            yield from gen_proj_munits(s, (4, 0))
            yield from gen_v_units(s)

        def gen_proj_tail(s):   # pg1-3 deps: fillered into chunk s itself
            yield from gen_proj_munits(s, (1, 2, 3))

        def interleave(*gens):
            gens = [g for g in gens if g is not None]
            i = 0
            while gens:
                g = gens[i % len(gens)]
                try:
                    next(g)
                except StopIteration:
                    gens.remove(g)
                    continue
                yield
                i += 1

        def winterleave(specs):
            """Round-robin with weights: (gen, weight) pulls weight
            units per round. Tail-proj gets weight 2 so qkrot[m] for
            upcoming head-pairs lands ahead of their first scores."""
            active = [(g, w) for g, w in specs if g is not None]
            while active:
                nxt = []
                for g, w in active:
                    alive = True
                    for _ in range(w):
                        try:
                            next(g)
                        except StopIteration:
                            alive = False
                            break
                        yield
                    if alive:
                        nxt.append((g, w))
                active = nxt

        # ---- proj head of chunk 0 dense (nothing to overlap with) ----
        with nc.named_scope("projA0"):
            for _ in gen_proj_head(0):
                pass

        # deferred o-unit generators: chunk c may drain earlier chunks'
        # o units up to a per-chunk cap; the rest spill to later chunks
        o_gens = []
        o_caps = {0: 0, 1: 16, 2: 16, 3: 16}

        def gen_o_budget(cap):
            spent = 0
            while o_gens and spent < cap:
                try:
                    next(o_gens[0])
                except StopIteration:
                    o_gens.pop(0)
                    continue
                spent += 1
                yield

        # ---- attention chunks; proj tail(c) + head(c+1) + deferred o
        # drip into the kb loop as PE fill work while ScalarE streams
        # exps ----
        for c in range(NSC):
          with nc.named_scope(f"attn_c{c}"):
            q0 = c * SC
            nkb = 4 * c + 4
            if c >= 1:
                o_gens.append(gen_o_chunk(c - 1))
            filler = winterleave([
                (gen_proj_tail(c), 2),
                (gen_proj_head(c + 1) if c + 1 < NSC else None, 1),
                (gen_o_budget(o_caps[c]), 1),
            ])
            n_units = 9 + (10 if c + 1 < NSC else 0) + (16 if c >= 1 else 0)
            total_iters = 4 * nkb
            it = 0
            spent = 0
            for pg in (0, 1, 2, 3):
                av0 = psum.tile([P, SC], F32, tag="av", bufs=2, name="av0")
                av1 = psum.tile([P, SC], F32, tag="av", bufs=2, name="av1")
                hp = pg

                def emit_av(kb, pt, vs):
                    nc.tensor.matmul(
                        av0[0:65, vs:SC],
                        lhsT=vnat[kb][:, 0:65],
                        rhs=pt[:, vs:SC],
                        start=(kb == 0), stop=(kb == nkb - 1),
                    )
                    nc.tensor.matmul(
                        av1[:, vs:SC],
                        lhsT=vnat[kb][:, 64:192],
                        rhs=pt[:, SC + vs:2 * SC],
                        start=(kb == 0), stop=(kb == nkb - 1),
                    )

                # software pipeline: AV(kb-3) is emitted after scores(kb),
                # giving each exp ~two iterations of cover
                pending = []
                for kb in range(nkb):
                    vs = max(0, (kb - 4 * c) * P)  # first valid col in chunk
                    st = psum.tile([P, 2 * SC], F32, tag="st", bufs=2, name="st")
                    nc.tensor.matmul(
                        st[:, vs:SC],
                        lhsT=qkrot[4][0:64, kb * P:(kb + 1) * P],
                        rhs=qkrot[hp][0:64, q0 + vs:q0 + SC],
                        start=True, stop=True,
                    )
                    nc.tensor.matmul(
                        st[:, SC + vs:2 * SC],
                        lhsT=qkrot[4][64:128, kb * P:(kb + 1) * P],
                        rhs=qkrot[hp][64:128, q0 + vs:q0 + SC],
                        start=True, stop=True,
                    )
                    if len(pending) >= 3:
                        emit_av(*pending.pop(0))
                    pt = ptp.tile([P, 2 * SC], BF16, tag="pt", name="pt")
                    # one exp over [vs:1024]: the dead span [SC:SC+vs] is
                    # unwritten PSUM (may exp to junk; never read)
                    nc.scalar.activation(
                        pt[:, vs:2 * SC], st[:, vs:2 * SC], EXP, scale=0.125
                    )
                    if kb - 4 * c >= 0:  # diagonal block: mask triangle
                        nc.vector.tensor_mul(
                            pt[:, vs:vs + P], pt[:, vs:vs + P], mask_sb
                        )
                        nc.gpsimd.tensor_mul(
                            pt[:, SC + vs:SC + vs + P],
                            pt[:, SC + vs:SC + vs + P], mask_sb
                        )
                    pending.append((kb, pt, vs))
                    it += 1
                    want = (it * n_units) // total_iters
                    while spent < want:
                        try:
                            next(filler)
                            spent += 1
                        except StopIteration:
                            spent = want
                            break
                for pp in pending:
                    emit_av(*pp)
                    try:
                        next(filler)
                        spent += 1
                    except StopIteration:
                        pass

                # ---- normalize: evict avs to SBUF first (frees the av
                # PSUM banks for the next pg's AV accumulation), then
                # den->recip->broadcast->mul chains off SBUF. B-chain
                # (den1 at partition 0, no DMA) is emitted first so its
                # ops run while dA's DMA is in flight.
                av0e = nrm.tile([65, SC], F32, tag="av0e", bufs=2, name="av0e")
                nc.vector.tensor_copy(av0e, av0[0:65, :])
                av1e = nrm.tile([P, SC], F32, tag="av1e", bufs=2, name="av1e")
                nc.vector.tensor_copy(av1e, av1)
                # DMA-broadcast both dens (0-stride src partition dim):
                # den0 -> dn[0:64], den1 -> dn[64:128]; one reciprocal
                # covers both heads; muls are lane-aligned with attnT.
                dn = nrm.tile([P, SC], F32, tag="dn", bufs=2, name="dn")
                nc.sync.dma_start(
                    out=dn[0:64, :],
                    in_=av0e[64:65, None, :].broadcast_to([1, 64, SC]),
                )
                nc.sync.dma_start(
                    out=dn[64:128, :],
                    in_=av1e[0:1, None, :].broadcast_to([1, 64, SC]),
                )
                rc = nrm.tile([P, SC], F32, tag="rc", bufs=2, name="rc")
                nc.vector.reciprocal_approx_fast(rc, dn)
                nc.vector.tensor_mul(
                    attnT[hp][0:64, q0:q0 + SC], av0e[0:64, :], rc[0:64, :]
                )
                nc.vector.tensor_mul(
                    attnT[hp][64:128, q0:q0 + SC], av1e[64:128, :], rc[64:128, :]
                )

            # drain remaining filler (next chunk depends on its qkrot/vnat)
            for _ in filler:
                pass
        # remaining deferred o units, then the last chunk's o_proj tail
        for g in o_gens:
            for _ in g:
                pass
        for _ in gen_o_chunk(NSC - 1):
            pass

    nc.finalize()
    return nc


def _pack16(a, rows):
    """[rows*128, N] -> [128, rows*N] (row-chunk-major columns)."""
    n = a.shape[1]
    return np.ascontiguousarray(
        a.reshape(rows, P, n).transpose(1, 0, 2).reshape(P, rows * n)
    )


def prep_core_inputs(x, cos, sin, wq, wk, wv, wo, core, _shared={}):
    """Build the per-core input map (all host-side numpy)."""
    b, g = core // 4, core % 4
    S = x.shape[1]
    NHID = HID // P
    NSC = S // SC

    key = ("xTp", b, id(x))
    if key not in _shared:
        _shared.clear() if len(_shared) > 8 else None
        # [128, NHID*S] with column layout (s-chunk c, hid chunk h, s')
        xT = x[b].T.astype(NP_BF16)  # [HID, S]
        a = xT.reshape(NHID, P, NSC, SC).transpose(1, 2, 0, 3)
        _shared[key] = np.ascontiguousarray(a.reshape(P, NHID * S))
    xTp = _shared[key]

    qcols = []
    for i in range(4):
        h0, h1 = 8 * g + i, 8 * g + i + 4
        qcols.append(wq[:, h0 * D:(h0 + 1) * D])
        qcols.append(wq[:, h1 * D:(h1 + 1) * D])
    kcols = wk[:, 2 * g * D:(2 * g + 2) * D]
    vcols = wv[:, 2 * g * D:(2 * g + 2) * D]
    # m-major blocks in consumption order: k, q-pair0, v, q-pairs 1-3
    blocks = [kcols, np.concatenate(qcols[0:2], axis=1), vcols,
              np.concatenate(qcols[2:4], axis=1),
              np.concatenate(qcols[4:6], axis=1),
              np.concatenate(qcols[6:8], axis=1)]
    wqkvp = np.concatenate(
        [_pack16(b.astype(NP_BF16), NHID) for b in blocks], axis=1)
    worows = []
    for i in range(4):
        h0, h1 = 8 * g + i, 8 * g + i + 4
        worows.append(wo[h0 * D:(h0 + 1) * D, :])
        worows.append(wo[h1 * D:(h1 + 1) * D, :])
    wo_c = np.concatenate(worows, axis=0).astype(NP_BF16)
    wop = _pack16(wo_c, 4)                  # [128, 4*HID]

    cosT = np.tile(cos[:S].T, (2, 1)).astype(NP_BF16)
    sinT_h = np.concatenate([-sin[:S].T[:D // 2], sin[:S].T[D // 2:]], axis=0)
    sinT = np.tile(sinT_h, (2, 1)).astype(NP_BF16)
    trimask = np.triu(np.ones((P, P), dtype=NP_BF16))
    csm = np.ascontiguousarray(
        np.concatenate([cosT, sinT, trimask], axis=1)
    )

    return {"xTp": xTp, "wqkvp": wqkvp, "csm": csm, "wop": wop}


def kernel(x, cos, sin, wq, wk, wv, wo):
    x = np.asarray(x)
    S = x.shape[1]
    assert x.shape == (B, S, HID)
    if S not in _CACHE:
        _CACHE[S] = build_nc(S)
    nc = _CACHE[S]
    in_maps = [
        prep_core_inputs(x, np.asarray(cos), np.asarray(sin), np.asarray(wq),
                         np.asarray(wk), np.asarray(wv), np.asarray(wo), core)
        for core in range(8)
    ]
    res = run_bass_kernel_spmd(nc, in_maps, core_ids=list(range(8)))
    out = np.zeros((B, S, HID), np.float32)
    for core in range(8):
        out[core // 4] += res.results[core]["o_part"].astype(np.float32)
    return out


# revision 14
# speedup vs baseline: 1.7190x; 1.0135x over previous
"""GQA attention (RoPE + causal softmax + o_proj) on 8 Trainium2 NeuronCores.

Sharding: core = b*4 + g where b = batch (2), g = head-group (4).
Each core handles 8 query heads (global 8g..8g+7) and their 2 KV heads
(2g, 2g+1) for one batch element, producing a partial o_proj output
(contraction over its 512 of the 2048 hd dims). The host sums the 4
partials per batch element (o_part is bf16; host upcasts).

Per-core layout/schedule (all matmul operands bf16, fp32 PSUM accum):
  - Inputs are host-packed into 4 flat [128, N] tensors so the whole
    input load is 5 DMA instructions, issued from sync/scalar/vector
    queues in parallel (DMA issue costs ~0.7us of the issuing engine's
    queue REGARDLESS of size, so many small input DMAs serialize the
    sync queue and block the rope-swap DMAs behind them).
  - q^T/k^T built per 128-row chunk pairing heads (i, i+4); scores are
    computed transposed (S^T[k,q]) as two row-tiled K=64 matmuls that
    run concurrently in the PE array.
  - AV stationary vnat[kb] = [v0(0:64) | 1 | 0*63 | v1(128:192)]:
      av0 = vnat[:,0:65].T  @ pt0 -> v0 at partitions 0:64, den0 at 64
      av1 = vnat[:,64:192].T @ pt1 -> den1 at partition 0, v1 at 64:128
    so attnT rows 64:128 are written lane-aligned (no SBUF->SBUF DMA)
    and den1 feeds reciprocal/broadcast without a partition move.
  - At pg end avs are evicted to SBUF immediately (frees the 2 "av"
    PSUM banks for the next pg), then den->recip->broadcast->mul chains
    run off SBUF (muls on GPSIMD; recip/evict on DVE).
  - Schedule: proj(0) dense, then attention chunk c interleaves
    proj(c+1) + o_proj(c-1) units as PE filler (own "fil" PSUM slots)
    so ScalarE exp (the per-kb rate limiter) starts early and PE never
    drains. PSUM: st 2x2 + av 2 + fil 2 = 8 banks.
  - Engine balance: exp exclusively on ACT; PSUM reads (qraw/t1/
    evictions) on DVE; rope t2+add, diag-mask half, norm muls on GPSIMD.
"""

import numpy as np
import ml_dtypes
from contextlib import ExitStack

import concourse.mybir as mybir
from concourse import bacc
from concourse.tile import TileContext
from concourse.bass_utils import run_bass_kernel_spmd

BF16 = mybir.dt.bfloat16
F32 = mybir.dt.float32
NP_BF16 = ml_dtypes.bfloat16

HID = 2048
D = 64
H = 32           # global query heads
KV = 8           # global kv heads
B = 2
P = 128
SC = 512         # q-chunk width (also matmul free dim / PSUM bank)

_CACHE = {}


def build_nc(S):
    assert S % SC == 0
    NHID = HID // P       # hid chunks (16)
    NSB = S // P          # 128-row s-blocks
    NSC = S // SC         # 512-col s-chunks
    QCH = 4               # q chunk-pairs
    EXP = mybir.ActivationFunctionType.Exp

    nc = bacc.Bacc("TRN2", target_bir_lowering=False, debug=False)
    # host-packed flat inputs (see prep_core_inputs for layouts)
    xTp = nc.dram_tensor("xTp", [P, NHID * S], BF16, kind="ExternalInput")
    wqkvp = nc.dram_tensor("wqkvp", [P, NHID * 768], BF16, kind="ExternalInput")
    csm = nc.dram_tensor("csm", [P, 2 * S + P], BF16, kind="ExternalInput")
    wop = nc.dram_tensor("wop", [P, 4 * HID], BF16, kind="ExternalInput")
    o_part = nc.dram_tensor("o_part", [S, HID], BF16, kind="ExternalOutput")

    with TileContext(nc) as tc, ExitStack() as ctx:
        res = ctx.enter_context(tc.tile_pool(name="res", bufs=1))
        rope = ctx.enter_context(tc.tile_pool(name="rope", bufs=3))
        ptp = ctx.enter_context(tc.tile_pool(name="ptp", bufs=5))
        nrm = ctx.enter_context(tc.tile_pool(name="nrm", bufs=2))
        obp = ctx.enter_context(tc.tile_pool(name="obp", bufs=2))
        psum = ctx.enter_context(tc.tile_pool(name="psum", bufs=1, space="PSUM"))

        # ---- input DMA: purpose-split queues. sync: xT chunk 0
        # (h-progressive quarters); scalar: weights m-major-progressive;
        # gpsimd: cos/sin/mask (small, needed by the first rope). The
        # rope-swap / den / o_part DMAs later share these queues by
        # latency class.
        xtall = res.tile([P, NHID * S], BF16, tag="xtall", name="xtall")
        wqall = res.tile([P, NHID * 768], BF16, tag="wqall", name="wqall")
        csm_t = res.tile([P, 2 * S + P], BF16, tag="csm", name="csm")
        nc.scalar.dma_start(out=csm_t, in_=csm[:, :])
        for q in range(4):      # xT c0: h-quarter q arrives early
            xsl = slice(q * 4 * SC, (q + 1) * 4 * SC)
            nc.sync.dma_start(out=xtall[:, xsl], in_=xTp[:, xsl])
        for b in range(6):      # weight blocks in consumption order
            wsl = slice(b * NHID * P, (b + 1) * NHID * P)
            nc.scalar.dma_start(out=wqall[:, wsl], in_=wqkvp[:, wsl])
        nc.scalar.dma_start(
            out=xtall[:, NHID * SC:], in_=xTp[:, NHID * SC:]
        )
        woall = res.tile([P, 4 * HID], BF16, tag="woall", name="woall")
        nc.scalar.dma_start(out=woall, in_=wop[:, :])

        # packed-layout views
        def xtv(h, s):      # x^T [hid chunk h, s-chunk s] -> [128, 512]
            base = (s * NHID + h) * SC
            return xtall[:, base:base + SC]

        def xtvb(h, sb):    # x^T [hid chunk h, s-block sb] -> [128, 128]
            base = ((sb // 4) * NHID + h) * SC + (sb % 4) * P
            return xtall[:, base:base + P]

        # wqall column layout: 6 blocks x 16 h x 128 cols, block order
        # (k, q-pair0, v, q-pair1, q-pair2, q-pair3) = consumption order
        _WBLK = {4: 0, 0: 1, 1: 3, 2: 4, 3: 5}

        def wqv(h, m):      # wqkv [hid chunk h, col chunk m] (m=4 -> k)
            base = (_WBLK[m] * NHID + h) * P
            return wqall[:, base:base + P]

        def wvv(h):         # wv [hid chunk h] -> [128, 128]
            base = (2 * NHID + h) * P
            return wqall[:, base:base + P]

        cos_sb = csm_t[:, 0:S]
        sin_sb = csm_t[:, S:2 * S]
        mask_sb = csm_t[:, 2 * S:2 * S + P]

        def wov(i, n):      # wo [hd chunk i, hid cols n*SC..] -> [128, 512]
            return woall[:, i * HID + n * SC: i * HID + (n + 1) * SC]

        # chunks 0-3: q head pairs (i, i+4); chunk 4: k (kv0 rows 0-63, kv1 64-127)
        qkrot = []
        for m in range(5):
            t = res.tile([P, S], BF16, tag=f"qkrot{m}", name=f"qkrot{m}")
            qkrot.append(t)
        # v tiles [128, 192]: [v0(0:64) | 1 | 0*63 | v1(128:192)]
        vnat = [res.tile([P, 192], BF16, tag=f"vnat{sb}", name=f"vnat{sb}")
                for sb in range(NSB)]
        attnT = []
        for i in range(QCH):
            t = res.tile([P, S], BF16, tag=f"attnT{i}", name=f"attnT{i}")
            attnT.append(t)

        def gen_o_chunk(c):
            for qb in range(4 * c, 4 * c + 4):
                ob = obp.tile([P, HID], BF16, tag="ob", name="ob")
                for n in range(HID // SC):
                    po = psum.tile([P, SC], F32, tag="fil", bufs=2, name="po")
                    for i in range(QCH):
                        nc.tensor.matmul(
                            po,
                            lhsT=attnT[i][:, qb * P:(qb + 1) * P],
                            rhs=wov(i, n),
                            start=(i == 0),
                            stop=(i == QCH - 1),
                        )
                    nc.vector.tensor_copy(ob[:, n * SC:(n + 1) * SC], po)
                    yield
                nc.scalar.dma_start(out=o_part[qb * P:(qb + 1) * P, :], in_=ob)

        def gen_proj_munits(s, ms):
            """Emit projection m-chunk units for s-chunk s (yields).

            Matmul groups stay consecutive; eviction units only follow
            completed groups. Rope eviction split: qraw/t1 on DVE (the
            PSUM readers), t2 + final add on GPSIMD so the DVE FIFO
            never head-of-line-blocks on the qswp DMA chain."""
            sl = slice(s * SC, (s + 1) * SC)
            for m in ms:
                ps = psum.tile([P, SC], F32, tag="fil", bufs=2, name="ps_proj")
                for h0 in (0, 8):
                    for h in range(h0, h0 + 8):
                        nc.tensor.matmul(
                            ps,
                            lhsT=wqv(h, m),
                            rhs=xtv(h, s),
                            start=(h == 0),
                            stop=(h == NHID - 1),
                        )
                    yield
                # rotate_half operand: engines are lane-locked, so the
                # +-32-partition swap must go through DMA (SBUF->SBUF);
                # one DMA per direction with a 3-level partition pattern
                qraw = rope.tile([P, SC], BF16, tag="qraw", bufs=3, name="qraw")
                nc.vector.tensor_copy(qraw, ps)
                qswp = rope.tile([P, SC], BF16, tag="qswp", bufs=3, name="qswp")
                for dst, srcp in ((0, 32), (32, 0), (64, 96), (96, 64)):
                    nc.sync.dma_start(
                        out=qswp[dst:dst + 32, :], in_=qraw[srcp:srcp + 32, :]
                    )
                t1 = rope.tile([P, SC], BF16, tag="t1", bufs=3, name="t1")
                nc.vector.tensor_mul(t1, ps, cos_sb[:, sl])
                nc.gpsimd.tensor_mul(qswp, qswp, sin_sb[:, sl])
                nc.gpsimd.tensor_add(qkrot[m][:, sl], t1, qswp)
                yield

        def gen_v_units(s):
            for sb in range(4 * s, 4 * s + 4):
                t = vnat[sb]
                nc.gpsimd.memset(t[:, 64:65], 1.0)
                nc.gpsimd.memset(t[:, 65:128], 0.0)
                pv = psum.tile([P, 128], F32, tag="fil", bufs=2, name="ps_v")
                for h in range(NHID):
                    nc.tensor.matmul(
                        pv,
                        lhsT=xtvb(h, sb),
                        rhs=wvv(h),
                        start=(h == 0),
                        stop=(h == NHID - 1),
                    )
                yield
                nc.vector.tensor_copy(t[:, 0:64], pv[:, 0:64])
                nc.vector.tensor_copy(t[:, 128:192], pv[:, 64:128])
                yield

        def gen_proj_head(s):   # pg0 deps of chunk s: k, q-pair 0, v
            yield from gen_proj_munits(s, (4, 0))
            yield from gen_v_units(s)

        def gen_proj_tail(s):   # pg1-3 deps: fillered into chunk s itself
            yield from gen_proj_munits(s, (1, 2, 3))

        def interleave(*gens):
            gens = [g for g in gens if g is not None]
            i = 0
            while gens:
                g = gens[i % len(gens)]
                try:
                    next(g)
                except StopIteration:
                    gens.remove(g)
                    continue
                yield
                i += 1

        def winterleave(specs):
            """Round-robin with weights: (gen, weight) pulls weight
            units per round. Tail-proj gets weight 2 so qkrot[m] for
            upcoming head-pairs lands ahead of their first scores."""
            active = [(g, w) for g, w in specs if g is not None]
            while active:
                nxt = []
                for g, w in active:
                    alive = True
                    for _ in range(w):
                        try:
                            next(g)
                        except StopIteration:
                            alive = False
                            break
                        yield
                    if alive:
                        nxt.append((g, w))
                active = nxt

        # ---- proj head of chunk 0 dense (nothing to overlap with) ----
        with nc.named_scope("projA0"):
            for _ in gen_proj_head(0):
                pass

        # deferred o-unit generators: chunk c may drain earlier chunks'
        # o units up to a per-chunk cap; the rest spill to later chunks
        o_gens = []
        o_caps = {0: 0, 1: 16, 2: 16, 3: 16}

        def gen_o_budget(cap):
            spent = 0
            while o_gens and spent < cap:
                try:
                    next(o_gens[0])
                except StopIteration:
                    o_gens.pop(0)
                    continue
                spent += 1
                yield

        # ---- attention chunks; proj tail(c) + head(c+1) + deferred o
        # drip into the kb loop as PE fill work while ScalarE streams
        # exps ----
        for c in range(NSC):
          with nc.named_scope(f"attn_c{c}"):
            q0 = c * SC
            nkb = 4 * c + 4
            if c >= 1:
                o_gens.append(gen_o_chunk(c - 1))
            filler = winterleave([
                (gen_proj_tail(c), 2),
                (gen_proj_head(c + 1) if c + 1 < NSC else None, 1),
                (gen_o_budget(o_caps[c]), 1),
            ])
            n_units = 9 + (10 if c + 1 < NSC else 0) + (16 if c >= 1 else 0)
            total_iters = 4 * nkb
            it = 0
            spent = 0
            for pg in (0, 1, 2, 3):
                av0 = psum.tile([P, SC], F32, tag="av", bufs=2, name="av0")
                av1 = psum.tile([P, SC], F32, tag="av", bufs=2, name="av1")
                hp = pg

                def emit_av(kb, pt, vs):
                    nc.tensor.matmul(
                        av0[0:65, vs:SC],
                        lhsT=vnat[kb][:, 0:65],
                        rhs=pt[:, vs:SC],
                        start=(kb == 0), stop=(kb == nkb - 1),
                    )
                    nc.tensor.matmul(
                        av1[:, vs:SC],
                        lhsT=vnat[kb][:, 64:192],
                        rhs=pt[:, SC + vs:2 * SC],
                        start=(kb == 0), stop=(kb == nkb - 1),
                    )

                # software pipeline: AV(kb-3) is emitted after scores(kb),
                # giving each exp ~two iterations of cover
                pending = []
                for kb in range(nkb):
                    vs = max(0, (kb - 4 * c) * P)  # first valid col in chunk
                    st = psum.tile([P, 2 * SC], F32, tag="st", bufs=2, name="st")
                    nc.tensor.matmul(
                        st[:, vs:SC],
                        lhsT=qkrot[4][0:64, kb * P:(kb + 1) * P],
                        rhs=qkrot[hp][0:64, q0 + vs:q0 + SC],
                        start=True, stop=True,
                    )
                    nc.tensor.matmul(
                        st[:, SC + vs:2 * SC],
                        lhsT=qkrot[4][64:128, kb * P:(kb + 1) * P],
                        rhs=qkrot[hp][64:128, q0 + vs:q0 + SC],
                        start=True, stop=True,
                    )
                    if len(pending) >= 3:
                        emit_av(*pending.pop(0))
                    pt = ptp.tile([P, 2 * SC], BF16, tag="pt", name="pt")
                    # one exp over [vs:1024]: the dead span [SC:SC+vs] is
                    # unwritten PSUM (may exp to junk; never read)
                    nc.scalar.activation(
                        pt[:, vs:2 * SC], st[:, vs:2 * SC], EXP, scale=0.125
                    )
                    if kb - 4 * c >= 0:  # diagonal block: mask triangle
                        nc.vector.tensor_mul(
                            pt[:, vs:vs + P], pt[:, vs:vs + P], mask_sb
                        )
                        nc.gpsimd.tensor_mul(
                            pt[:, SC + vs:SC + vs + P],
                            pt[:, SC + vs:SC + vs + P], mask_sb
                        )
                    pending.append((kb, pt, vs))
                    it += 1
                    want = (it * n_units) // total_iters
                    while spent < want:
                        try:
                            next(filler)
                            spent += 1
                        except StopIteration:
                            spent = want
                            break
                for pp in pending:
                    emit_av(*pp)
                    try:
                        next(filler)
                        spent += 1
                    except StopIteration:
                        pass

                # ---- normalize: evict avs to SBUF first (frees the av
                # PSUM banks for the next pg's AV accumulation), then
                # den->recip->broadcast->mul chains off SBUF. B-chain
                # (den1 at partition 0, no DMA) is emitted first so its
                # ops run while dA's DMA is in flight.
                av0e = nrm.tile([65, SC], F32, tag="av0e", bufs=2, name="av0e")
                nc.vector.tensor_copy(av0e, av0[0:65, :])
                av1e = nrm.tile([P, SC], F32, tag="av1e", bufs=2, name="av1e")
                nc.vector.tensor_copy(av1e, av1)
                # DMA-broadcast both dens (0-stride src partition dim):
                # den0 -> dn[0:64], den1 -> dn[64:128]; one reciprocal
                # covers both heads; muls are lane-aligned with attnT.
                dn = nrm.tile([P, SC], F32, tag="dn", bufs=2, name="dn")
                nc.sync.dma_start(
                    out=dn[0:64, :],
                    in_=av0e[64:65, None, :].broadcast_to([1, 64, SC]),
                )
                nc.sync.dma_start(
                    out=dn[64:128, :],
                    in_=av1e[0:1, None, :].broadcast_to([1, 64, SC]),
                )
                rc = nrm.tile([P, SC], F32, tag="rc", bufs=2, name="rc")
                nc.vector.reciprocal_approx_fast(rc, dn)
                nc.vector.tensor_mul(
                    attnT[hp][0:64, q0:q0 + SC], av0e[0:64, :], rc[0:64, :]
                )
                nc.vector.tensor_mul(
                    attnT[hp][64:128, q0:q0 + SC], av1e[64:128, :], rc[64:128, :]
                )

            # drain remaining filler (next chunk depends on its qkrot/vnat)
            for _ in filler:
                pass
        # remaining deferred o units, then the last chunk's o_proj tail
        for g in o_gens:
            for _ in g:
                pass
        for _ in gen_o_chunk(NSC - 1):
            pass

    nc.finalize()
    return nc


def _pack16(a, rows):
    """[rows*128, N] -> [128, rows*N] (row-chunk-major columns)."""
    n = a.shape[1]
    return np.ascontiguousarray(
        a.reshape(rows, P, n).transpose(1, 0, 2).reshape(P, rows * n)
    )


def prep_core_inputs(x, cos, sin, wq, wk, wv, wo, core, _shared={}):
    """Build the per-core input map (all host-side numpy)."""
    b, g = core // 4, core % 4
    S = x.shape[1]
    NHID = HID // P
    NSC = S // SC

    key = ("xTp", b, id(x))
    if key not in _shared:
        _shared.clear() if len(_shared) > 8 else None
        # [128, NHID*S] with column layout (s-chunk c, hid chunk h, s')
        xT = x[b].T.astype(NP_BF16)  # [HID, S]
        a = xT.reshape(NHID, P, NSC, SC).transpose(1, 2, 0, 3)
        _shared[key] = np.ascontiguousarray(a.reshape(P, NHID * S))
    xTp = _shared[key]

    qcols = []
    for i in range(4):
        h0, h1 = 8 * g + i, 8 * g + i + 4
        qcols.append(wq[:, h0 * D:(h0 + 1) * D])
        qcols.append(wq[:, h1 * D:(h1 + 1) * D])
    kcols = wk[:, 2 * g * D:(2 * g + 2) * D]
    vcols = wv[:, 2 * g * D:(2 * g + 2) * D]
    # m-major blocks in consumption order: k, q-pair0, v, q-pairs 1-3
    blocks = [kcols, np.concatenate(qcols[0:2], axis=1), vcols,
              np.concatenate(qcols[2:4], axis=1),
              np.concatenate(qcols[4:6], axis=1),
              np.concatenate(qcols[6:8], axis=1)]
    wqkvp = np.concatenate(
        [_pack16(b.astype(NP_BF16), NHID) for b in blocks], axis=1)
    worows = []
    for i in range(4):
        h0, h1 = 8 * g + i, 8 * g + i + 4
        worows.append(wo[h0 * D:(h0 + 1) * D, :])
        worows.append(wo[h1 * D:(h1 + 1) * D, :])
    wo_c = np.concatenate(worows, axis=0).astype(NP_BF16)
    wop = _pack16(wo_c, 4)                  # [128, 4*HID]

    cosT = np.tile(cos[:S].T, (2, 1)).astype(NP_BF16)
    sinT_h = np.concatenate([-sin[:S].T[:D // 2], sin[:S].T[D // 2:]], axis=0)
    sinT = np.tile(sinT_h, (2, 1)).astype(NP_BF16)
    trimask = np.triu(np.ones((P, P), dtype=NP_BF16))
    csm = np.ascontiguousarray(
        np.concatenate([cosT, sinT, trimask], axis=1)
    )

    return {"xTp": xTp, "wqkvp": wqkvp, "csm": csm, "wop": wop}


def kernel(x, cos, sin, wq, wk, wv, wo):
    x = np.asarray(x)
    S = x.shape[1]
    assert x.shape == (B, S, HID)
    if S not in _CACHE:
        _CACHE[S] = build_nc(S)
    nc = _CACHE[S]
    in_maps = [
        prep_core_inputs(x, np.asarray(cos), np.asarray(sin), np.asarray(wq),
                         np.asarray(wk), np.asarray(wv), np.asarray(wo), core)
        for core in range(8)
    ]
    res = run_bass_kernel_spmd(nc, in_maps, core_ids=list(range(8)))
    out = np.zeros((B, S, HID), np.float32)
    for core in range(8):
        out[core // 4] += res.results[core]["o_part"].astype(np.float32)
    return out


# revision 15
# speedup vs baseline: 1.7902x; 1.0415x over previous
"""GQA attention (RoPE + causal softmax + o_proj) on 8 Trainium2 NeuronCores.

Sharding: core = b*4 + g where b = batch (2), g = head-group (4).
Each core handles 8 query heads (global 8g..8g+7) and their 2 KV heads
(2g, 2g+1) for one batch element, producing a partial o_proj output
(contraction over its 512 of the 2048 hd dims). The host sums the 4
partials per batch element (o_part is bf16; host upcasts).

Per-core layout/schedule (all matmul operands bf16, fp32 PSUM accum):
  - Inputs are host-packed into 4 flat [128, N] tensors so the whole
    input load is 5 DMA instructions, issued from sync/scalar/vector
    queues in parallel (DMA issue costs ~0.7us of the issuing engine's
    queue REGARDLESS of size, so many small input DMAs serialize the
    sync queue and block the rope-swap DMAs behind them).
  - q^T/k^T built per 128-row chunk pairing heads (i, i+4); scores are
    computed transposed (S^T[k,q]) as two row-tiled K=64 matmuls that
    run concurrently in the PE array.
  - AV stationary vnat[kb] = [v0(0:64) | 1 | 0*63 | v1(128:192)]:
      av0 = vnat[:,0:65].T  @ pt0 -> v0 at partitions 0:64, den0 at 64
      av1 = vnat[:,64:192].T @ pt1 -> den1 at partition 0, v1 at 64:128
    so attnT rows 64:128 are written lane-aligned (no SBUF->SBUF DMA)
    and den1 feeds reciprocal/broadcast without a partition move.
  - At pg end avs are evicted to SBUF immediately (frees the 2 "av"
    PSUM banks for the next pg), then den->recip->broadcast->mul chains
    run off SBUF (muls on GPSIMD; recip/evict on DVE).
  - Schedule: proj(0) dense, then attention chunk c interleaves
    proj(c+1) + o_proj(c-1) units as PE filler (own "fil" PSUM slots)
    so ScalarE exp (the per-kb rate limiter) starts early and PE never
    drains. PSUM: st 2x2 + av 2 + fil 2 = 8 banks.
  - Engine balance: exp exclusively on ACT; PSUM reads (qraw/t1/
    evictions) on DVE; rope t2+add, diag-mask half, norm muls on GPSIMD.
"""

import numpy as np
import ml_dtypes
from contextlib import ExitStack

import concourse.mybir as mybir
from concourse import bacc
from concourse.tile import TileContext
from concourse.bass_utils import run_bass_kernel_spmd

BF16 = mybir.dt.bfloat16
F32 = mybir.dt.float32
NP_BF16 = ml_dtypes.bfloat16

HID = 2048
D = 64
H = 32           # global query heads
KV = 8           # global kv heads
B = 2
P = 128
SC = 512         # q-chunk width (also matmul free dim / PSUM bank)

_CACHE = {}


def build_nc(S):
    assert S % SC == 0
    NHID = HID // P       # hid chunks (16)
    NSB = S // P          # 128-row s-blocks
    NSC = S // SC         # 512-col s-chunks
    QCH = 4               # q chunk-pairs
    EXP = mybir.ActivationFunctionType.Exp

    nc = bacc.Bacc("TRN2", target_bir_lowering=False, debug=False)
    # host-packed flat inputs (see prep_core_inputs for layouts)
    xTp = nc.dram_tensor("xTp", [P, NHID * S], BF16, kind="ExternalInput")
    wqkvp = nc.dram_tensor("wqkvp", [P, NHID * 768], BF16, kind="ExternalInput")
    csm = nc.dram_tensor("csm", [P, 2 * S + 2 * P], BF16, kind="ExternalInput")
    wop = nc.dram_tensor("wop", [P, 4 * HID], BF16, kind="ExternalInput")
    o_part = nc.dram_tensor("o_part", [S, HID], BF16, kind="ExternalOutput")

    with TileContext(nc) as tc, ExitStack() as ctx:
        res = ctx.enter_context(tc.tile_pool(name="res", bufs=1))
        rope = ctx.enter_context(tc.tile_pool(name="rope", bufs=3))
        ptp = ctx.enter_context(tc.tile_pool(name="ptp", bufs=5))
        nrm = ctx.enter_context(tc.tile_pool(name="nrm", bufs=2))
        obp = ctx.enter_context(tc.tile_pool(name="obp", bufs=2))
        psum = ctx.enter_context(tc.tile_pool(name="psum", bufs=1, space="PSUM"))

        # ---- input DMA: purpose-split queues. sync: xT chunk 0
        # (h-progressive quarters); scalar: weights m-major-progressive;
        # gpsimd: cos/sin/mask (small, needed by the first rope). The
        # rope-swap / den / o_part DMAs later share these queues by
        # latency class.
        xtall = res.tile([P, NHID * S], BF16, tag="xtall", name="xtall")
        wqall = res.tile([P, NHID * 768], BF16, tag="wqall", name="wqall")
        csm_t = res.tile([P, 2 * S + 2 * P], BF16, tag="csm", name="csm")
        for q in range(4):      # xT c0: h-quarter q arrives early
            xsl = slice(q * 4 * SC, (q + 1) * 4 * SC)
            nc.sync.dma_start(out=xtall[:, xsl], in_=xTp[:, xsl])
        for b in range(6):      # weight blocks in consumption order
            wsl = slice(b * NHID * P, (b + 1) * NHID * P)
            nc.scalar.dma_start(out=wqall[:, wsl], in_=wqkvp[:, wsl])
            if b == 0:
                nc.scalar.dma_start(out=csm_t, in_=csm[:, :])
        nc.scalar.dma_start(
            out=xtall[:, NHID * SC:], in_=xTp[:, NHID * SC:]
        )
        woall = res.tile([P, 4 * HID], BF16, tag="woall", name="woall")
        nc.scalar.dma_start(out=woall, in_=wop[:, :])

        # packed-layout views
        def xtv(h, s):      # x^T [hid chunk h, s-chunk s] -> [128, 512]
            base = (s * NHID + h) * SC
            return xtall[:, base:base + SC]

        def xtvb(h, sb):    # x^T [hid chunk h, s-block sb] -> [128, 128]
            base = ((sb // 4) * NHID + h) * SC + (sb % 4) * P
            return xtall[:, base:base + P]

        # wqall column layout: 6 blocks x 16 h x 128 cols, block order
        # (k, q-pair0, v, q-pair1, q-pair2, q-pair3) = consumption order
        _WBLK = {4: 0, 0: 1, 1: 3, 2: 4, 3: 5}

        def wqv(h, m):      # wqkv [hid chunk h, col chunk m] (m=4 -> k)
            base = (_WBLK[m] * NHID + h) * P
            return wqall[:, base:base + P]

        def wvv(h):         # wv [hid chunk h] -> [128, 128]
            base = (2 * NHID + h) * P
            return wqall[:, base:base + P]

        cos_sb = csm_t[:, 0:S]
        sin_sb = csm_t[:, S:2 * S]
        mask_sb = csm_t[:, 2 * S:2 * S + P]
        perm_sb = csm_t[:, 2 * S + P:2 * S + 2 * P]   # rotate-half swap

        def wov(i, n):      # wo [hd chunk i, hid cols n*SC..] -> [128, 512]
            return woall[:, i * HID + n * SC: i * HID + (n + 1) * SC]

        # chunks 0-3: q head pairs (i, i+4); chunk 4: k (kv0 rows 0-63, kv1 64-127)
        qkrot = []
        for m in range(5):
            t = res.tile([P, S], BF16, tag=f"qkrot{m}", name=f"qkrot{m}")
            qkrot.append(t)
        # v tiles [128, 192]: [v0(0:64) | 1 | 0*63 | v1(128:192)]
        vnat = [res.tile([P, 192], BF16, tag=f"vnat{sb}", name=f"vnat{sb}")
                for sb in range(NSB)]
        attnT = []
        for i in range(QCH):
            t = res.tile([P, S], BF16, tag=f"attnT{i}", name=f"attnT{i}")
            attnT.append(t)

        def gen_o_chunk(c):
            for qb in range(4 * c, 4 * c + 4):
                ob = obp.tile([P, HID], BF16, tag="ob", name="ob")
                for n in range(HID // SC):
                    po = psum.tile([P, SC], F32, tag="fil", bufs=2, name="po")
                    for i in range(QCH):
                        nc.tensor.matmul(
                            po,
                            lhsT=attnT[i][:, qb * P:(qb + 1) * P],
                            rhs=wov(i, n),
                            start=(i == 0),
                            stop=(i == QCH - 1),
                        )
                    nc.vector.tensor_copy(ob[:, n * SC:(n + 1) * SC], po)
                    yield
                nc.scalar.dma_start(out=o_part[qb * P:(qb + 1) * P, :], in_=ob)

        def gen_proj_munits(s, ms):
            """Emit projection m-chunk units for s-chunk s (yields).

            Matmul groups stay consecutive; eviction units only follow
            completed groups. Rope eviction split: qraw/t1 on DVE (the
            PSUM readers), t2 + final add on GPSIMD so the DVE FIFO
            never head-of-line-blocks on the qswp DMA chain."""
            sl = slice(s * SC, (s + 1) * SC)
            for m in ms:
                ps = psum.tile([P, SC], F32, tag="fil", bufs=2, name="ps_proj")
                for h0 in (0, 8):
                    for h in range(h0, h0 + 8):
                        nc.tensor.matmul(
                            ps,
                            lhsT=wqv(h, m),
                            rhs=xtv(h, s),
                            start=(h == 0),
                            stop=(h == NHID - 1),
                        )
                    yield
                # rotate_half: engines are lane-locked, so the +-32-
                # partition swap runs on the PE as a constant permutation
                # matmul (no DMA; sign lives in sinT host-side)
                qraw = rope.tile([P, SC], BF16, tag="qraw", bufs=3, name="qraw")
                nc.vector.tensor_copy(qraw, ps)
                t1 = rope.tile([P, SC], BF16, tag="t1", bufs=3, name="t1")
                nc.vector.tensor_mul(t1, ps, cos_sb[:, sl])
                ps2 = psum.tile([P, SC], F32, tag="fil", bufs=2, name="ps_rot")
                nc.tensor.matmul(ps2, lhsT=perm_sb, rhs=qraw,
                                 start=True, stop=True)
                t2 = rope.tile([P, SC], BF16, tag="t2", bufs=3, name="t2")
                nc.vector.tensor_mul(t2, ps2, sin_sb[:, sl])
                nc.gpsimd.tensor_add(qkrot[m][:, sl], t1, t2)
                yield

        def gen_v_units(s):
            for sb in range(4 * s, 4 * s + 4):
                t = vnat[sb]
                nc.gpsimd.memset(t[:, 64:65], 1.0)
                nc.gpsimd.memset(t[:, 65:128], 0.0)
                pv = psum.tile([P, 128], F32, tag="fil", bufs=2, name="ps_v")
                for h in range(NHID):
                    nc.tensor.matmul(
                        pv,
                        lhsT=xtvb(h, sb),
                        rhs=wvv(h),
                        start=(h == 0),
                        stop=(h == NHID - 1),
                    )
                yield
                nc.vector.tensor_copy(t[:, 0:64], pv[:, 0:64])
                nc.vector.tensor_copy(t[:, 128:192], pv[:, 64:128])
                yield

        def gen_proj_head(s):   # pg0 deps of chunk s: k, q-pair 0, v
            yield from gen_proj_munits(s, (4, 0))
            yield from gen_v_units(s)

        def gen_proj_tail(s):   # pg1-3 deps: fillered into chunk s itself
            yield from gen_proj_munits(s, (1, 2, 3))

        def interleave(*gens):
            gens = [g for g in gens if g is not None]
            i = 0
            while gens:
                g = gens[i % len(gens)]
                try:
                    next(g)
                except StopIteration:
                    gens.remove(g)
                    continue
                yield
                i += 1

        def winterleave(specs):
            """Round-robin with weights: (gen, weight) pulls weight
            units per round. Tail-proj gets weight 2 so qkrot[m] for
            upcoming head-pairs lands ahead of their first scores."""
            active = [(g, w) for g, w in specs if g is not None]
            while active:
                nxt = []
                for g, w in active:
                    alive = True
                    for _ in range(w):
                        try:
                            next(g)
                        except StopIteration:
                            alive = False
                            break
                        yield
                    if alive:
                        nxt.append((g, w))
                active = nxt

        # ---- proj head of chunk 0 dense (nothing to overlap with) ----
        with nc.named_scope("projA0"):
            for _ in gen_proj_head(0):
                pass

        # deferred o-unit generators: chunk c may drain earlier chunks'
        # o units up to a per-chunk cap; the rest spill to later chunks
        o_gens = []
        o_caps = {0: 0, 1: 16, 2: 16, 3: 16}

        def gen_o_budget(cap):
            spent = 0
            while o_gens and spent < cap:
                try:
                    next(o_gens[0])
                except StopIteration:
                    o_gens.pop(0)
                    continue
                spent += 1
                yield

        # ---- attention chunks; proj tail(c) + head(c+1) + deferred o
        # drip into the kb loop as PE fill work while ScalarE streams
        # exps ----
        for c in range(NSC):
          with nc.named_scope(f"attn_c{c}"):
            q0 = c * SC
            nkb = 4 * c + 4
            if c >= 1:
                o_gens.append(gen_o_chunk(c - 1))
            filler = winterleave([
                (gen_proj_tail(c), 2),
                (gen_proj_head(c + 1) if c + 1 < NSC else None, 1),
                (gen_o_budget(o_caps[c]), 1),
            ])
            n_units = 9 + (10 if c + 1 < NSC else 0) + (16 if c >= 1 else 0)
            total_iters = 4 * nkb
            it = 0
            spent = 0
            for pg in (0, 1, 2, 3):
                av0 = psum.tile([P, SC], F32, tag="av", bufs=2, name="av0")
                av1 = psum.tile([P, SC], F32, tag="av", bufs=2, name="av1")
                hp = pg

                def emit_av(kb, pt, vs):
                    nc.tensor.matmul(
                        av0[0:65, vs:SC],
                        lhsT=vnat[kb][:, 0:65],
                        rhs=pt[:, vs:SC],
                        start=(kb == 0), stop=(kb == nkb - 1),
                    )
                    nc.tensor.matmul(
                        av1[:, vs:SC],
                        lhsT=vnat[kb][:, 64:192],
                        rhs=pt[:, SC + vs:2 * SC],
                        start=(kb == 0), stop=(kb == nkb - 1),
                    )

                # software pipeline: AV(kb-3) is emitted after scores(kb),
                # giving each exp ~two iterations of cover
                pending = []
                for kb in range(nkb):
                    vs = max(0, (kb - 4 * c) * P)  # first valid col in chunk
                    st = psum.tile([P, 2 * SC], F32, tag="st", bufs=2, name="st")
                    nc.tensor.matmul(
                        st[:, vs:SC],
                        lhsT=qkrot[4][0:64, kb * P:(kb + 1) * P],
                        rhs=qkrot[hp][0:64, q0 + vs:q0 + SC],
                        start=True, stop=True,
                    )
                    nc.tensor.matmul(
                        st[:, SC + vs:2 * SC],
                        lhsT=qkrot[4][64:128, kb * P:(kb + 1) * P],
                        rhs=qkrot[hp][64:128, q0 + vs:q0 + SC],
                        start=True, stop=True,
                    )
                    if len(pending) >= 3:
                        emit_av(*pending.pop(0))
                    pt = ptp.tile([P, 2 * SC], BF16, tag="pt", name="pt")
                    # one exp over [vs:1024]: the dead span [SC:SC+vs] is
                    # unwritten PSUM (may exp to junk; never read)
                    nc.scalar.activation(
                        pt[:, vs:2 * SC], st[:, vs:2 * SC], EXP, scale=0.125
                    )
                    if kb - 4 * c >= 0:  # diagonal block: mask triangle
                        nc.vector.tensor_mul(
                            pt[:, vs:vs + P], pt[:, vs:vs + P], mask_sb
                        )
                        nc.gpsimd.tensor_mul(
                            pt[:, SC + vs:SC + vs + P],
                            pt[:, SC + vs:SC + vs + P], mask_sb
                        )
                    pending.append((kb, pt, vs))
                    it += 1
                    want = (it * n_units) // total_iters
                    while spent < want:
                        try:
                            next(filler)
                            spent += 1
                        except StopIteration:
                            spent = want
                            break
                for pp in pending:
                    emit_av(*pp)
                    try:
                        next(filler)
                        spent += 1
                    except StopIteration:
                        pass

                # ---- normalize: evict avs to SBUF first (frees the av
                # PSUM banks for the next pg's AV accumulation), then
                # den->recip->broadcast->mul chains off SBUF. B-chain
                # (den1 at partition 0, no DMA) is emitted first so its
                # ops run while dA's DMA is in flight.
                av0e = nrm.tile([65, SC], F32, tag="av0e", bufs=2, name="av0e")
                nc.vector.tensor_copy(av0e, av0[0:65, :])
                av1e = nrm.tile([P, SC], F32, tag="av1e", bufs=2, name="av1e")
                nc.vector.tensor_copy(av1e, av1)
                # DMA-broadcast both dens (0-stride src partition dim):
                # den0 -> dn[0:64], den1 -> dn[64:128]; one reciprocal
                # covers both heads; muls are lane-aligned with attnT.
                dn = nrm.tile([P, SC], F32, tag="dn", bufs=2, name="dn")
                nc.sync.dma_start(
                    out=dn[0:64, :],
                    in_=av0e[64:65, None, :].broadcast_to([1, 64, SC]),
                )
                nc.sync.dma_start(
                    out=dn[64:128, :],
                    in_=av1e[0:1, None, :].broadcast_to([1, 64, SC]),
                )
                rc = nrm.tile([P, SC], F32, tag="rc", bufs=2, name="rc")
                nc.vector.reciprocal_approx_fast(rc, dn)
                nc.vector.tensor_mul(
                    attnT[hp][0:64, q0:q0 + SC], av0e[0:64, :], rc[0:64, :]
                )
                nc.vector.tensor_mul(
                    attnT[hp][64:128, q0:q0 + SC], av1e[64:128, :], rc[64:128, :]
                )

            # drain remaining filler (next chunk depends on its qkrot/vnat)
            for _ in filler:
                pass
        # remaining deferred o units, then the last chunk's o_proj tail
        for g in o_gens:
            for _ in g:
                pass
        for _ in gen_o_chunk(NSC - 1):
            pass

    nc.finalize()
    return nc


def _pack16(a, rows):
    """[rows*128, N] -> [128, rows*N] (row-chunk-major columns)."""
    n = a.shape[1]
    return np.ascontiguousarray(
        a.reshape(rows, P, n).transpose(1, 0, 2).reshape(P, rows * n)
    )


def prep_core_inputs(x, cos, sin, wq, wk, wv, wo, core, _shared={}):
    """Build the per-core input map (all host-side numpy)."""
    b, g = core // 4, core % 4
    S = x.shape[1]
    NHID = HID // P
    NSC = S // SC

    key = ("xTp", b, id(x))
    if key not in _shared:
        _shared.clear() if len(_shared) > 8 else None
        # [128, NHID*S] with column layout (s-chunk c, hid chunk h, s')
        xT = x[b].T.astype(NP_BF16)  # [HID, S]
        a = xT.reshape(NHID, P, NSC, SC).transpose(1, 2, 0, 3)
        _shared[key] = np.ascontiguousarray(a.reshape(P, NHID * S))
    xTp = _shared[key]

    qcols = []
    for i in range(4):
        h0, h1 = 8 * g + i, 8 * g + i + 4
        qcols.append(wq[:, h0 * D:(h0 + 1) * D])
        qcols.append(wq[:, h1 * D:(h1 + 1) * D])
    kcols = wk[:, 2 * g * D:(2 * g + 2) * D]
    vcols = wv[:, 2 * g * D:(2 * g + 2) * D]
    # m-major blocks in consumption order: k, q-pair0, v, q-pairs 1-3
    blocks = [kcols, np.concatenate(qcols[0:2], axis=1), vcols,
              np.concatenate(qcols[2:4], axis=1),
              np.concatenate(qcols[4:6], axis=1),
              np.concatenate(qcols[6:8], axis=1)]
    wqkvp = np.concatenate(
        [_pack16(b.astype(NP_BF16), NHID) for b in blocks], axis=1)
    worows = []
    for i in range(4):
        h0, h1 = 8 * g + i, 8 * g + i + 4
        worows.append(wo[h0 * D:(h0 + 1) * D, :])
        worows.append(wo[h1 * D:(h1 + 1) * D, :])
    wo_c = np.concatenate(worows, axis=0).astype(NP_BF16)
    wop = _pack16(wo_c, 4)                  # [128, 4*HID]

    cosT = np.tile(cos[:S].T, (2, 1)).astype(NP_BF16)
    sinT_h = np.concatenate([-sin[:S].T[:D // 2], sin[:S].T[D // 2:]], axis=0)
    sinT = np.tile(sinT_h, (2, 1)).astype(NP_BF16)
    trimask = np.triu(np.ones((P, P), dtype=NP_BF16))
    perm = np.zeros((P, P), dtype=NP_BF16)
    for j in range(P):
        base = (j // 64) * 64
        perm[base + ((j - base + 32) % 64), j] = 1
    csm = np.ascontiguousarray(
        np.concatenate([cosT, sinT, trimask, perm], axis=1)
    )

    return {"xTp": xTp, "wqkvp": wqkvp, "csm": csm, "wop": wop}


def kernel(x, cos, sin, wq, wk, wv, wo):
    x = np.asarray(x)
    S = x.shape[1]
    assert x.shape == (B, S, HID)
    if S not in _CACHE:
        _CACHE[S] = build_nc(S)
    nc = _CACHE[S]
    in_maps = [
        prep_core_inputs(x, np.asarray(cos), np.asarray(sin), np.asarray(wq),
                         np.asarray(wk), np.asarray(wv), np.asarray(wo), core)
        for core in range(8)
    ]
    res = run_bass_kernel_spmd(nc, in_maps, core_ids=list(range(8)))
    out = np.zeros((B, S, HID), np.float32)
    for core in range(8):
        out[core // 4] += res.results[core]["o_part"].astype(np.float32)
    return out


# revision 17
# speedup vs baseline: 2.0549x; 1.1479x over previous
"""GQA attention (RoPE + causal softmax + o_proj) on 8 Trainium2 NeuronCores.

Sharding: core = b*4 + g where b = batch (2), g = head-group (4).
Each core handles 8 query heads (global 8g..8g+7) and their 2 KV heads
(2g, 2g+1) for one batch element, producing a partial o_proj output
(contraction over its 512 of the 2048 hd dims). The host sums the 4
partials per batch element (o_part is bf16; host upcasts).

Per-core layout/schedule (all matmul operands bf16, fp32 PSUM accum):
  - Inputs are host-packed into 4 flat [128, N] tensors so the whole
    input load is 5 DMA instructions, issued from sync/scalar/vector
    queues in parallel (DMA issue costs ~0.7us of the issuing engine's
    queue REGARDLESS of size, so many small input DMAs serialize the
    sync queue and block the rope-swap DMAs behind them).
  - q^T/k^T built per 128-row chunk pairing heads (i, i+4); scores are
    computed transposed (S^T[k,q]) as two row-tiled K=64 matmuls that
    run concurrently in the PE array.
  - AV stationary vnat[kb] = [v0(0:64) | 1 | 0*63 | v1(128:192)]:
      av0 = vnat[:,0:65].T  @ pt0 -> v0 at partitions 0:64, den0 at 64
      av1 = vnat[:,64:192].T @ pt1 -> den1 at partition 0, v1 at 64:128
    so attnT rows 64:128 are written lane-aligned (no SBUF->SBUF DMA)
    and den1 feeds reciprocal/broadcast without a partition move.
  - At pg end avs are evicted to SBUF immediately (frees the 2 "av"
    PSUM banks for the next pg), then den->recip->broadcast->mul chains
    run off SBUF (muls on GPSIMD; recip/evict on DVE).
  - Schedule: proj(0) dense, then attention chunk c interleaves
    proj(c+1) + o_proj(c-1) units as PE filler (own "fil" PSUM slots)
    so ScalarE exp (the per-kb rate limiter) starts early and PE never
    drains. PSUM: st 2x2 + av 2 + fil 2 = 8 banks.
  - Engine balance: exp exclusively on ACT; PSUM reads (qraw/t1/
    evictions) on DVE; rope t2+add, diag-mask half, norm muls on GPSIMD.
"""

import numpy as np
import ml_dtypes
from contextlib import ExitStack

import concourse.mybir as mybir
from concourse import bacc
from concourse.tile import TileContext
from concourse.bass_utils import run_bass_kernel_spmd

BF16 = mybir.dt.bfloat16
F32 = mybir.dt.float32
NP_BF16 = ml_dtypes.bfloat16

HID = 2048
D = 64
H = 32           # global query heads
KV = 8           # global kv heads
B = 2
P = 128
SC = 512         # q-chunk width (also matmul free dim / PSUM bank)

_CACHE = {}


def build_nc(S):
    assert S % SC == 0
    NHID = HID // P       # hid chunks (16)
    NSB = S // P          # 128-row s-blocks
    NSC = S // SC         # 512-col s-chunks
    QCH = 4               # q chunk-pairs
    EXP = mybir.ActivationFunctionType.Exp

    nc = bacc.Bacc("TRN2", target_bir_lowering=False, debug=False)
    # host-packed flat inputs (see prep_core_inputs for layouts)
    xTp = nc.dram_tensor("xTp", [P, NHID * S], BF16, kind="ExternalInput")
    wqkvp = nc.dram_tensor("wqkvp", [P, NHID * 768], BF16, kind="ExternalInput")
    csm = nc.dram_tensor("csm", [P, 2 * S + 3 * P], BF16, kind="ExternalInput")
    wop = nc.dram_tensor("wop", [P, 4 * HID], BF16, kind="ExternalInput")
    o_part = nc.dram_tensor("o_part", [S, HID], BF16, kind="ExternalOutput")

    with TileContext(nc) as tc, ExitStack() as ctx:
        res = ctx.enter_context(tc.tile_pool(name="res", bufs=1))
        rope = ctx.enter_context(tc.tile_pool(name="rope", bufs=3))
        ptp = ctx.enter_context(tc.tile_pool(name="ptp", bufs=5))
        nrm = ctx.enter_context(tc.tile_pool(name="nrm", bufs=2))
        obp = ctx.enter_context(tc.tile_pool(name="obp", bufs=2))
        psum = ctx.enter_context(tc.tile_pool(name="psum", bufs=1, space="PSUM"))

        # ---- input DMA: purpose-split queues. sync: xT chunk 0
        # (h-progressive quarters); scalar: weights m-major-progressive;
        # gpsimd: cos/sin/mask (small, needed by the first rope). The
        # rope-swap / den / o_part DMAs later share these queues by
        # latency class.
        xtall = res.tile([P, NHID * S], BF16, tag="xtall", name="xtall")
        wqall = res.tile([P, NHID * 768], BF16, tag="wqall", name="wqall")
        csm_t = res.tile([P, 2 * S + 3 * P], BF16, tag="csm", name="csm")
        for q in range(4):      # xT c0: h-quarter q arrives early
            xsl = slice(q * 4 * SC, (q + 1) * 4 * SC)
            nc.sync.dma_start(out=xtall[:, xsl], in_=xTp[:, xsl])
        for b in range(6):      # weight blocks in consumption order
            wsl = slice(b * NHID * P, (b + 1) * NHID * P)
            nc.scalar.dma_start(out=wqall[:, wsl], in_=wqkvp[:, wsl])
            if b == 0:
                nc.scalar.dma_start(out=csm_t, in_=csm[:, :])
        nc.scalar.dma_start(
            out=xtall[:, NHID * SC:], in_=xTp[:, NHID * SC:]
        )
        woall = res.tile([P, 4 * HID], BF16, tag="woall", name="woall")
        nc.scalar.dma_start(out=woall, in_=wop[:, :])

        # packed-layout views
        def xtv(h, s):      # x^T [hid chunk h, s-chunk s] -> [128, 512]
            base = (s * NHID + h) * SC
            return xtall[:, base:base + SC]

        def xtvb(h, sb):    # x^T [hid chunk h, s-block sb] -> [128, 128]
            base = ((sb // 4) * NHID + h) * SC + (sb % 4) * P
            return xtall[:, base:base + P]

        # wqall column layout: 6 blocks x 16 h x 128 cols, block order
        # (k, q-pair0, v, q-pair1, q-pair2, q-pair3) = consumption order
        _WBLK = {4: 0, 0: 1, 1: 3, 2: 4, 3: 5}

        def wqv(h, m):      # wqkv [hid chunk h, col chunk m] (m=4 -> k)
            base = (_WBLK[m] * NHID + h) * P
            return wqall[:, base:base + P]

        def wvv(h):         # wv [hid chunk h] -> [128, 128]
            base = (2 * NHID + h) * P
            return wqall[:, base:base + P]

        cos_sb = csm_t[:, 0:S]
        sin_sb = csm_t[:, S:2 * S]
        mask_sb = csm_t[:, 2 * S:2 * S + P]
        perm_sb = csm_t[:, 2 * S + P:2 * S + 2 * P]   # rotate-half swap
        ones_sb = csm_t[:, 2 * S + 2 * P:2 * S + 3 * P]

        def wov(i, n):      # wo [hd chunk i, hid cols n*SC..] -> [128, 512]
            return woall[:, i * HID + n * SC: i * HID + (n + 1) * SC]

        # chunks 0-3: q head pairs (i, i+4); chunk 4: k (kv0 rows 0-63, kv1 64-127)
        qkrot = []
        for m in range(5):
            t = res.tile([P, S], BF16, tag=f"qkrot{m}", name=f"qkrot{m}")
            qkrot.append(t)
        # v tiles [128, 192]: [v0(0:64) | 1 | 0*63 | v1(128:192)]
        vnat = [res.tile([P, 192], BF16, tag=f"vnat{sb}", name=f"vnat{sb}")
                for sb in range(NSB)]
        attnT = []
        for i in range(QCH):
            t = res.tile([P, S], BF16, tag=f"attnT{i}", name=f"attnT{i}")
            attnT.append(t)

        def gen_o_chunk(c):
            for qb in range(4 * c, 4 * c + 4):
                ob = obp.tile([P, HID], BF16, tag="ob", name="ob")
                for n in range(HID // SC):
                    po = psum.tile([P, SC], F32, tag="fil", bufs=2, name="po")
                    for i in range(QCH):
                        nc.tensor.matmul(
                            po,
                            lhsT=attnT[i][:, qb * P:(qb + 1) * P],
                            rhs=wov(i, n),
                            start=(i == 0),
                            stop=(i == QCH - 1),
                        )
                    nc.vector.tensor_copy(ob[:, n * SC:(n + 1) * SC], po)
                    yield
                nc.scalar.dma_start(out=o_part[qb * P:(qb + 1) * P, :], in_=ob)

        def gen_proj_munits(s, ms):
            """Emit projection m-chunk units for s-chunk s (yields).

            Matmul groups stay consecutive; eviction units only follow
            completed groups. Rope eviction split: qraw/t1 on DVE (the
            PSUM readers), t2 + final add on GPSIMD so the DVE FIFO
            never head-of-line-blocks on the qswp DMA chain."""
            sl = slice(s * SC, (s + 1) * SC)
            for m in ms:
                ps = psum.tile([P, SC], F32, tag="fil", bufs=2, name="ps_proj")
                for h0 in (0, 8):
                    for h in range(h0, h0 + 8):
                        nc.tensor.matmul(
                            ps,
                            lhsT=wqv(h, m),
                            rhs=xtv(h, s),
                            start=(h == 0),
                            stop=(h == NHID - 1),
                        )
                    yield
                # rotate_half: engines are lane-locked, so the +-32-
                # partition swap runs on the PE as a constant permutation
                # matmul (no DMA; sign lives in sinT host-side)
                qraw = rope.tile([P, SC], BF16, tag="qraw", bufs=3, name="qraw")
                nc.vector.tensor_copy(qraw, ps)
                t1 = rope.tile([P, SC], BF16, tag="t1", bufs=3, name="t1")
                nc.vector.tensor_mul(t1, ps, cos_sb[:, sl])
                ps2 = psum.tile([P, SC], F32, tag="fil", bufs=2, name="ps_rot")
                nc.tensor.matmul(ps2, lhsT=perm_sb, rhs=qraw,
                                 start=True, stop=True)
                t2 = rope.tile([P, SC], BF16, tag="t2", bufs=3, name="t2")
                nc.vector.tensor_mul(t2, ps2, sin_sb[:, sl])
                nc.gpsimd.tensor_add(qkrot[m][:, sl], t1, t2)
                yield

        def gen_v_units(s):
            for sb in range(4 * s, 4 * s + 4):
                t = vnat[sb]
                nc.gpsimd.memset(t[:, 64:65], 1.0)
                nc.gpsimd.memset(t[:, 65:128], 0.0)
                pv = psum.tile([P, 128], F32, tag="fil", bufs=2, name="ps_v")
                for h in range(NHID):
                    nc.tensor.matmul(
                        pv,
                        lhsT=xtvb(h, sb),
                        rhs=wvv(h),
                        start=(h == 0),
                        stop=(h == NHID - 1),
                    )
                yield
                nc.vector.tensor_copy(t[:, 0:64], pv[:, 0:64])
                nc.vector.tensor_copy(t[:, 128:192], pv[:, 64:128])
                yield

        def gen_proj_head(s):   # pg0 deps of chunk s: k, q-pair 0, v
            yield from gen_proj_munits(s, (4, 0))
            yield from gen_v_units(s)

        def gen_proj_tail(s):   # pg1-3 deps: fillered into chunk s itself
            yield from gen_proj_munits(s, (1, 2, 3))

        def interleave(*gens):
            gens = [g for g in gens if g is not None]
            i = 0
            while gens:
                g = gens[i % len(gens)]
                try:
                    next(g)
                except StopIteration:
                    gens.remove(g)
                    continue
                yield
                i += 1

        def winterleave(specs):
            """Round-robin with weights: (gen, weight) pulls weight
            units per round. Tail-proj gets weight 2 so qkrot[m] for
            upcoming head-pairs lands ahead of their first scores."""
            active = [(g, w) for g, w in specs if g is not None]
            while active:
                nxt = []
                for g, w in active:
                    alive = True
                    for _ in range(w):
                        try:
                            next(g)
                        except StopIteration:
                            alive = False
                            break
                        yield
                    if alive:
                        nxt.append((g, w))
                active = nxt

        # ---- proj head of chunk 0 dense (nothing to overlap with) ----
        with nc.named_scope("projA0"):
            for _ in gen_proj_head(0):
                pass

        # deferred o-unit generators: chunk c may drain earlier chunks'
        # o units up to a per-chunk cap; the rest spill to later chunks
        o_gens = []
        o_caps = {0: 0, 1: 16, 2: 16, 3: 16}

        def gen_o_budget(cap):
            spent = 0
            while o_gens and spent < cap:
                try:
                    next(o_gens[0])
                except StopIteration:
                    o_gens.pop(0)
                    continue
                spent += 1
                yield

        # ---- attention chunks; proj tail(c) + head(c+1) + deferred o
        # drip into the kb loop as PE fill work while ScalarE streams
        # exps ----
        for c in range(NSC):
          with nc.named_scope(f"attn_c{c}"):
            q0 = c * SC
            nkb = 4 * c + 4
            if c >= 1:
                o_gens.append(gen_o_chunk(c - 1))
            filler = winterleave([
                (gen_proj_tail(c), 2),
                (gen_proj_head(c + 1) if c + 1 < NSC else None, 1),
                (gen_o_budget(o_caps[c]), 1),
            ])
            n_units = 9 + (10 if c + 1 < NSC else 0) + (16 if c >= 1 else 0)
            total_iters = 4 * nkb
            it = 0
            spent = 0
            for pg in (0, 1, 2, 3):
                av0 = psum.tile([P, SC], F32, tag="av", bufs=2, name="av0")
                av1 = psum.tile([P, SC], F32, tag="av", bufs=2, name="av1")
                hp = pg

                def emit_av(kb, pt, vs):
                    nc.tensor.matmul(
                        av0[0:65, vs:SC],
                        lhsT=vnat[kb][:, 0:65],
                        rhs=pt[:, vs:SC],
                        start=(kb == first_kb), stop=(kb == last_kb),
                    )
                    nc.tensor.matmul(
                        av1[:, vs:SC],
                        lhsT=vnat[kb][:, 64:192],
                        rhs=pt[:, SC + vs:2 * SC],
                        start=(kb == first_kb), stop=(kb == last_kb),
                    )

                # software pipeline: AV(kb-3) is emitted after scores(kb),
                # giving each exp ~two iterations of cover
                pending = []
                kb_order = list(range(4 * c, nkb)) + list(range(0, 4 * c))
                first_kb, last_kb = kb_order[0], kb_order[-1]
                for kb in kb_order:
                    vs = max(0, (kb - 4 * c) * P)  # first valid col in chunk
                    st = psum.tile([P, 2 * SC], F32, tag="st", bufs=2, name="st")
                    nc.tensor.matmul(
                        st[:, vs:SC],
                        lhsT=qkrot[4][0:64, kb * P:(kb + 1) * P],
                        rhs=qkrot[hp][0:64, q0 + vs:q0 + SC],
                        start=True, stop=True,
                    )
                    nc.tensor.matmul(
                        st[:, SC + vs:2 * SC],
                        lhsT=qkrot[4][64:128, kb * P:(kb + 1) * P],
                        rhs=qkrot[hp][64:128, q0 + vs:q0 + SC],
                        start=True, stop=True,
                    )
                    if len(pending) >= 3:
                        emit_av(*pending.pop(0))
                    pt = ptp.tile([P, 2 * SC], BF16, tag="pt", name="pt")
                    # one exp over [vs:1024]: the dead span [SC:SC+vs] is
                    # unwritten PSUM (may exp to junk; never read)
                    nc.scalar.activation(
                        pt[:, vs:2 * SC], st[:, vs:2 * SC], EXP, scale=0.125
                    )
                    if kb - 4 * c >= 0:  # diagonal block: mask triangle
                        nc.vector.tensor_mul(
                            pt[:, vs:vs + P], pt[:, vs:vs + P], mask_sb
                        )
                        nc.gpsimd.tensor_mul(
                            pt[:, SC + vs:SC + vs + P],
                            pt[:, SC + vs:SC + vs + P], mask_sb
                        )
                    pending.append((kb, pt, vs))
                    it += 1
                    want = (it * n_units) // total_iters
                    while spent < want:
                        try:
                            next(filler)
                            spent += 1
                        except StopIteration:
                            spent = want
                            break
                for pp in pending:
                    emit_av(*pp)
                    try:
                        next(filler)
                        spent += 1
                    except StopIteration:
                        pass

                # ---- normalize: evict avs to SBUF first (frees the av
                # PSUM banks for the next pg's AV accumulation), then
                # den->recip->broadcast->mul chains off SBUF. B-chain
                # (den1 at partition 0, no DMA) is emitted first so its
                # ops run while dA's DMA is in flight.
                av0e = nrm.tile([65, SC], BF16, tag="av0e", bufs=2, name="av0e")
                nc.vector.tensor_copy(av0e, av0[0:65, :])
                av1e = nrm.tile([P, SC], BF16, tag="av1e", bufs=2, name="av1e")
                nc.vector.tensor_copy(av1e, av1)
                # PE-broadcast both dens into one PSUM bank (two K=1
                # col-tiled matmuls with a constant ones stationary:
                # den0 -> partitions 0:64, den1 -> 64:128), then one
                # reciprocal covers both heads; muls are lane-aligned.
                dnp = psum.tile([P, SC], F32, tag="av", bufs=2, name="dnp")
                nc.tensor.matmul(dnp[0:64, :], lhsT=ones_sb[64:65, 0:64],
                                 rhs=av0e[64:65, :], start=True, stop=True)
                nc.tensor.matmul(dnp[64:128, :], lhsT=ones_sb[0:1, 64:128],
                                 rhs=av1e[0:1, :], start=True, stop=True)
                rc = nrm.tile([P, SC], F32, tag="rc", bufs=2, name="rc")
                nc.vector.reciprocal_approx_fast(rc, dnp)
                nc.vector.tensor_mul(
                    attnT[hp][0:64, q0:q0 + SC], av0e[0:64, :], rc[0:64, :]
                )
                nc.vector.tensor_mul(
                    attnT[hp][64:128, q0:q0 + SC], av1e[64:128, :], rc[64:128, :]
                )

            # drain remaining filler (next chunk depends on its qkrot/vnat)
            for _ in filler:
                pass
        # remaining deferred o units, then the last chunk's o_proj tail
        for g in o_gens:
            for _ in g:
                pass
        for _ in gen_o_chunk(NSC - 1):
            pass

    nc.finalize()
    return nc


def _pack16(a, rows):
    """[rows*128, N] -> [128, rows*N] (row-chunk-major columns)."""
    n = a.shape[1]
    return np.ascontiguousarray(
        a.reshape(rows, P, n).transpose(1, 0, 2).reshape(P, rows * n)
    )


def prep_core_inputs(x, cos, sin, wq, wk, wv, wo, core, _shared={}):
    """Build the per-core input map (all host-side numpy)."""
    b, g = core // 4, core % 4
    S = x.shape[1]
    NHID = HID // P
    NSC = S // SC

    key = ("xTp", b, id(x))
    if key not in _shared:
        _shared.clear() if len(_shared) > 8 else None
        # [128, NHID*S] with column layout (s-chunk c, hid chunk h, s')
        xT = x[b].T.astype(NP_BF16)  # [HID, S]
        a = xT.reshape(NHID, P, NSC, SC).transpose(1, 2, 0, 3)
        _shared[key] = np.ascontiguousarray(a.reshape(P, NHID * S))
    xTp = _shared[key]

    qcols = []
    for i in range(4):
        h0, h1 = 8 * g + i, 8 * g + i + 4
        qcols.append(wq[:, h0 * D:(h0 + 1) * D])
        qcols.append(wq[:, h1 * D:(h1 + 1) * D])
    kcols = wk[:, 2 * g * D:(2 * g + 2) * D]
    vcols = wv[:, 2 * g * D:(2 * g + 2) * D]
    # m-major blocks in consumption order: k, q-pair0, v, q-pairs 1-3
    blocks = [kcols, np.concatenate(qcols[0:2], axis=1), vcols,
              np.concatenate(qcols[2:4], axis=1),
              np.concatenate(qcols[4:6], axis=1),
              np.concatenate(qcols[6:8], axis=1)]
    wqkvp = np.concatenate(
        [_pack16(b.astype(NP_BF16), NHID) for b in blocks], axis=1)
    worows = []
    for i in range(4):
        h0, h1 = 8 * g + i, 8 * g + i + 4
        worows.append(wo[h0 * D:(h0 + 1) * D, :])
        worows.append(wo[h1 * D:(h1 + 1) * D, :])
    wo_c = np.concatenate(worows, axis=0).astype(NP_BF16)
    wop = _pack16(wo_c, 4)                  # [128, 4*HID]

    cosT = np.tile(cos[:S].T, (2, 1)).astype(NP_BF16)
    sinT_h = np.concatenate([-sin[:S].T[:D // 2], sin[:S].T[D // 2:]], axis=0)
    sinT = np.tile(sinT_h, (2, 1)).astype(NP_BF16)
    trimask = np.triu(np.ones((P, P), dtype=NP_BF16))
    perm = np.zeros((P, P), dtype=NP_BF16)
    for j in range(P):
        base = (j // 64) * 64
        perm[base + ((j - base + 32) % 64), j] = 1
    onesb = np.ones((P, P), dtype=NP_BF16)
    csm = np.ascontiguousarray(
        np.concatenate([cosT, sinT, trimask, perm, onesb], axis=1)
    )

    return {"xTp": xTp, "wqkvp": wqkvp, "csm": csm, "wop": wop}


def kernel(x, cos, sin, wq, wk, wv, wo):
    x = np.asarray(x)
    S = x.shape[1]
    assert x.shape == (B, S, HID)
    if S not in _CACHE:
        _CACHE[S] = build_nc(S)
    nc = _CACHE[S]
    in_maps = [
        prep_core_inputs(x, np.asarray(cos), np.asarray(sin), np.asarray(wq),
                         np.asarray(wk), np.asarray(wv), np.asarray(wo), core)
        for core in range(8)
    ]
    res = run_bass_kernel_spmd(nc, in_maps, core_ids=list(range(8)))
    out = np.zeros((B, S, HID), np.float32)
    for core in range(8):
        out[core // 4] += res.results[core]["o_part"].astype(np.float32)
    return out
